# revision 1
# baseline (speedup 1.0000x reference)
"""Trainium2 Bass kernel for nn_AttentionTwoStream (two-stream Bahdanau attention
with global gating softmax), distributed over 8 NeuronCores.

Sharding: data-parallel over batch B=64 -> 8 batches per core; all (512,512)
weights replicated. The only cross-core coupling is the beta softmax over
concat([mv1, mt1], axis=0) logits (2B = 128 scalars) -> a 16-float AllGather
(preceded by a warmup AllGather that hides the ncfw pipeline spin-up).

Compute: bf16 operands on the TensorEngine (1 cycle/row, halves HBM traffic),
fp32 PSUM accumulation, fp32 softmax/reductions on Vector/Scalar engines.

Self-contained: hardcodes shapes B=64, Tv=512, Tt=64, H=512, 8 cores.
"""

import numpy as np
import ml_dtypes

import concourse.bacc as bacc
import concourse.mybir as mybir
import concourse.tile as tile
from concourse.bass_utils import run_bass_kernel_spmd

NC = 8          # cores
B = 64          # global batch
BL = B // NC    # batches per core = 8
H = 512
Tv = 512
Tt = 64
KT = H // 128   # 4 contraction tiles
F32 = mybir.dt.float32
BF16 = mybir.dt.bfloat16
NP_BF16 = ml_dtypes.bfloat16


def build_nc():
    nc = bacc.Bacc(
        "TRN2", target_bir_lowering=False, debug=False,
        enable_asserts=False, num_devices=NC, num_swdge_queues=4,
    )

    def inp(name, shape, dt=BF16):
        return nc.dram_tensor(name, list(shape), dt, kind="ExternalInput").ap()

    # --- external inputs (per-core shards; layouts match SBUF tiles exactly) ---
    fT = inp("fT", (BL, 128, KT * Tv))       # frames^T  [b][p][kt*Tv+t], h=kt*128+p
    tT = inp("tT", (128, KT * BL * Tt))      # text^T    [p][kt*512 + b*64 + t]
    hT = inp("hT", (128, KT * BL))           # h^T       [p][kt*8+b]
    wav = inp("wav", (128, KT * H))          # Wav       [p][kt*512+j]  (k=kt*128+p)
    wat = inp("wat", (128, KT * H))
    uav = inp("uav", (128, KT * H))
    uat = inp("uat", (128, KT * H))
    whh = inp("whh", (128, KT * H))
    wb_m = inp("wb_m", (128, KT * H))        # Wb matrix
    wveT = inp("wveT", (128, KT * H))        # Wve.T packed
    wqeT = inp("wqeT", (128, KT * H))
    vbv = inp("vbv", (128, KT * H))
    vbt = inp("vbt", (128, KT * H))
    vavZ = inp("vavZ", (128, KT * 4))         # [p][jt*4+i*2+m]: col m==i -> Vav
    vatT = inp("vatT", (128, KT))            # Vat  [p][jt]
    biasr = inp("biasr", (1, 8 * H))         # rows: bav,bat,bve,bqe,bbv,bbt,bh,(pad)
    wbB = inp("wbB", (BL, H), F32)           # wb broadcast to 8 partitions
    eye = inp("eye", (128, 128))
    blkI = inp("blkI", (BL, BL * Tt))        # blkI[b, b'*64+t] = (b == b')

    out_ext = nc.dram_tensor("out", [BL, H], F32, kind="ExternalOutput").ap()

    ACT = mybir.ActivationFunctionType
    ALU = mybir.AluOpType

    with tile.TileContext(nc) as tc:
        with (
            tc.tile_pool(name="wres", bufs=1) as wres,       # resident tiles
            tc.tile_pool(name="wstream", bufs=3) as wstream, # streamed weights
            tc.tile_pool(name="work", bufs=4) as work,       # Y tiles etc
            tc.tile_pool(name="small", bufs=1) as small,
            tc.tile_pool(name="psX", bufs=3, space="PSUM") as psX,
            tc.tile_pool(name="psS", bufs=2, space="PSUM") as psS,
            tc.tile_pool(name="psB", bufs=1, space="PSUM") as psB,
            tc.tile_pool(name="psG", bufs=2, space="PSUM") as psG,
            tc.tile_pool(name="dram", bufs=1, space="DRAM") as dram,
        ):
            # ---------- DMAs (order matters for the critical path) ----------
            def load(pool, ap_in, shape, tag, dt=BF16, engine=None, name=None):
                t = pool.tile(list(shape), dt, tag=tag, name=name or tag)
                (engine or nc.sync).dma_start(t[:], ap_in)
                return t

            # tiny control tensors + P1 weights first
            hT_sb = load(wres, hT, (128, KT * BL), "hT")
            biasr_sb = load(wres, biasr, (1, 8 * H), "biasr")
            eye_sb = load(wres, eye, (128, 128), "eye")
            blkI_sb = load(wres, blkI, (BL, BL * Tt), "blkI")
            uav_sb = load(wstream, uav, (128, KT * H), "wstream", name="uav_t", engine=nc.gpsimd)
            uat_sb = load(wstream, uat, (128, KT * H), "wstream", name="uat_t", engine=nc.gpsimd)
            # text + frames weights on the scalar HWDGE queue (parallel issue)
            wat_sb = load(wres, wat, (128, KT * H), "wat", engine=nc.scalar)
            wav_sb = load(wres, wav, (128, KT * H), "wav", engine=nc.scalar)
            tT_sb = load(wres, tT, (128, KT * BL * Tt), "tT")
            vavZ_sb = load(wres, vavZ, (128, KT * 4), "vavZ")
            vatT_sb = load(wres, vatT, (128, KT), "vatT")
            wbB_sb = load(wres, wbB, (BL, H), "wbB", dt=F32)

            fT_sb = []
            for b in range(BL):
                t = wres.tile([128, KT * Tv], BF16, tag=f"fT{b}", name=f"fTs{b}")
                nc.gpsimd.dma_start(t[:], fT[b])
                fT_sb.append(t)

            ones_sb = small.tile([1, 128], BF16, tag="ones")
            nc.vector.memset(ones_sb[:], 1.0)

            def brow(i):
                return biasr_sb[0:1, i * H:(i + 1) * H]
            bav_r, bat_r, bve_r, bqe_r, bbv_r, bbt_r, bh_r = (brow(i) for i in range(7))

            # PE warmup: junk matmuls on eye while input DMAs land (HAM -> K=8/8)
            warm_ps = psB.tile([128, Tv], F32, tag="psB", name="warmps")
            for w in range(20):
                nc.tensor.matmul(
                    warm_ps[:], blkI_sb[0:BL, 0:128], blkI_sb[0:BL, :],
                    start=True, stop=True, skip_group_check=True,
                )

            # sum_kt lhs[:,kt*8:+8].T @ W[:,kt*512:+512] (+ ones x bias) -> [8,512]
            def gate_matmul(lhs_sb, w_sb, bias_row, ps=None, name="gps"):
                if ps is None:
                    ps = psG.tile([BL, H], F32, tag="psG", name=name)
                for kt in range(KT):
                    nc.tensor.matmul(
                        ps[:], lhs_sb[:, kt * BL:(kt + 1) * BL],
                        w_sb[:, kt * H:(kt + 1) * H],
                        start=(kt == 0), stop=False,
                        skip_group_check=True,
                    )
                nc.tensor.matmul(
                    ps[:], ones_sb[0:1, 0:BL], bias_row,
                    start=False, stop=True, skip_group_check=True,
                )
                return ps

            # ---------- P1: h-projections ----------
            uhvb_ps = gate_matmul(hT_sb, uav_sb, bav_r, name="uhvb")  # Uhv + bav
            uhtb_ps = gate_matmul(hT_sb, uat_sb, bat_r, name="uhtb")  # Uht + bat

            # evacuate to sbuf (bf16 rows for matmul reuse)
            uhvb_s = small.tile([BL, H], BF16, tag="uhvb_s")
            nc.scalar.copy(uhvb_s[:], uhvb_ps[:])
            uhtb_s = small.tile([BL, H], BF16, tag="uhtb_s")
            nc.scalar.copy(uhtb_s[:], uhtb_ps[:])

            # ---------- warmup collective (primes ncfw; result unused) ----------
            warm_out = dram.tile([2 * B, 1], F32, tag="warmout", addr_space="Shared")
            warm_in = dram.tile([2 * BL, 1], F32, tag="warmin")
            nc.gpsimd.collective_compute(
                "AllGather", ALU.bypass,
                replica_groups=[list(range(NC))],
                ins=[warm_in[:].opt()],
                outs=[warm_out[:].opt()],
            )

            # ---------- text stream: Xt = Wat.T@tT + Uht-bias, tanh, scores ----
            sct_ps = psS.tile([1, BL * Tt], F32, tag="scS", name="sct")
            pend_t = []

            def flush_sct():
                for yt_, jt_ in pend_t:
                    nc.tensor.matmul(
                        sct_ps[:], vatT_sb[:, jt_: jt_ + 1], yt_[:],
                        start=(jt_ == 0), stop=(jt_ == KT - 1),
                        skip_group_check=True,
                    )
                pend_t.clear()

            for jt in range(KT):
                xt_ps = psX.tile([128, BL * Tt], F32, tag="psX", name=f"xt{jt}")
                for kt in range(KT):
                    nc.tensor.matmul(
                        xt_ps[:],
                        wat_sb[:, kt * H + jt * 128: kt * H + (jt + 1) * 128],
                        tT_sb[:, kt * BL * Tt:(kt + 1) * BL * Tt],
                        start=(kt == 0), stop=False, skip_group_check=True,
                    )
                # bias add: out[j, b*64+t] += Uhtb[b, jt*128+j] via block-identity
                nc.tensor.matmul(
                    xt_ps[:], uhtb_s[0:BL, jt * 128:(jt + 1) * 128], blkI_sb[:],
                    start=False, stop=True, skip_group_check=True,
                )
                flush_sct()
                yt = work.tile([128, BL * Tt], BF16, tag="yt", name=f"yt{jt}")
                nc.scalar.activation(yt[:], xt_ps[:], ACT.Tanh)
                pend_t.append((yt, jt))
            flush_sct()

            # frames bias needs [512,8] layout for per-partition ACT bias
            uhvbT_sb = small.tile([128, KT * BL], F32, tag="uhvbT")
            for jt in range(KT):
                tp = psB.tile([128, BL], BF16, tag="psB", name=f"tpv{jt}")
                nc.tensor.transpose(
                    tp[:], uhvb_s[0:BL, jt * 128:(jt + 1) * 128],
                    eye_sb[0:BL, 0:BL],
                )
                nc.vector.tensor_copy(uhvbT_sb[:, jt * BL:(jt + 1) * BL], tp[:])

            # hWhh + bh
            whh_sb = load(wstream, whh, (128, KT * H), "wstream", name="whh_t", engine=nc.gpsimd)
            hwhh_ps = gate_matmul(hT_sb, whh_sb, bh_r, name="hwhh")
            hwhh_sb = small.tile([BL, H], F32, tag="hwhh_sb")
            nc.scalar.copy(hwhh_sb[:], hwhh_ps[:])

            # ---------- text softmax ----------
            sct_sb = small.tile([1, BL * Tt], F32, tag="sct_sb")
            nc.scalar.copy(sct_sb[:], sct_ps[:])
            st8 = small.tile([BL, Tt], F32, tag="st8")
            nc.sync.dma_start(st8[:, :], sct_sb[0:1, :])
            expt_sb = small.tile([BL, Tt], F32, tag="expt")
            sumt = small.tile([BL, 1], F32, tag="sumt")
            nc.scalar.activation(
                expt_sb[:], st8[:], ACT.Exp, accum_out=sumt[:],
            )
            rt = small.tile([BL, 1], F32, tag="rt")
            nc.vector.reciprocal(rt[:], sumt[:])
            at_sb = small.tile([BL, Tt], BF16, tag="at")
            nc.vector.tensor_scalar_mul(at_sb[:], expt_sb[:], rt[:])
            atRows = small.tile([1, BL * Tt], BF16, tag="atRows")
            nc.sync.dma_start(atRows[0:1, :], at_sb[:, :])

            # ---------- text einsum + gating, kt-outer with interleaved gates --
            wbm_sb = load(wstream, wb_m, (128, KT * H), "wstream", name="wbm_t", engine=nc.scalar)
            vbt_sb = load(wstream, vbt, (128, KT * H), "wstream", name="vbt_t", engine=nc.gpsimd)
            wqeT_sb = load(wstream, wqeT, (128, KT * H), "wstream", name="wqeT_t")

            atB_sb = []
            for b in range(BL):
                atB_ps = psB.tile([128, Tt], F32, tag="psB", name=f"atB{b}")
                src = at_sb[0:1, :] if b == 0 else atRows[0:1, b * Tt:(b + 1) * Tt]
                nc.tensor.matmul(
                    atB_ps[:], ones_sb[0:1, 0:128], src,
                    start=True, stop=True,
                )
                t = work.tile([128, Tt], BF16, tag="atB_sb", name=f"atBs{b}", bufs=8)
                nc.scalar.copy(t[:], atB_ps[:])
                atB_sb.append(t)

            # mt1/ht2: h@Wb and bias terms don't need htT -> accumulate first
            mt1_ps = psG.tile([BL, H], F32, tag="psG", name="mt1")
            gate_matmul(hT_sb, wbm_sb, bbt_r, ps=mt1_ps)   # h@Wb + bbt
            ht2_ps = psG.tile([BL, H], F32, tag="psG", name="ht2")
            nc.tensor.matmul(                               # just the bias for now
                ht2_ps[:], ones_sb[0:1, 0:BL], bqe_r,
                start=True, stop=False, skip_group_check=True,
            )

            htT_sb = small.tile([128, KT * BL], F32, tag="htT")
            htT_bf = small.tile([128, KT * BL], BF16, tag="htT_bf")
            for kt in range(KT):
                for b in range(BL):
                    scrt = work.tile([128, Tt], BF16, tag="scrt")
                    nc.vector.scalar_tensor_tensor(
                        out=scrt[:],
                        in0=tT_sb[:, kt * BL * Tt + b * Tt: kt * BL * Tt + (b + 1) * Tt],
                        scalar=1.0,
                        in1=atB_sb[b][:],
                        op0=ALU.mult, op1=ALU.mult,
                        accum_out=htT_sb[:, kt * BL + b: kt * BL + b + 1],
                    )
                nc.vector.tensor_copy(
                    htT_bf[:, kt * BL:(kt + 1) * BL],
                    htT_sb[:, kt * BL:(kt + 1) * BL],
                )
                nc.tensor.matmul(
                    mt1_ps[:], htT_bf[:, kt * BL:(kt + 1) * BL],
                    vbt_sb[:, kt * H:(kt + 1) * H],
                    start=False, stop=(kt == KT - 1), skip_group_check=True,
                )
                nc.tensor.matmul(
                    ht2_ps[:], htT_bf[:, kt * BL:(kt + 1) * BL],
                    wqeT_sb[:, kt * H:(kt + 1) * H],
                    start=False, stop=(kt == KT - 1), skip_group_check=True,
                )

            mtv_t = small.tile([BL, H], F32, tag="mtv_t")
            nc.scalar.activation(mtv_t[:], mt1_ps[:], ACT.Tanh)
            lgt = small.tile([BL, 1], F32, tag="lgt")
            scr8b = small.tile([BL, H], F32, tag="scr8b")
            nc.vector.scalar_tensor_tensor(
                out=scr8b[:], in0=mtv_t[:], scalar=1.0, in1=wbB_sb[:],
                op0=ALU.mult, op1=ALU.mult, accum_out=lgt[:],
            )
            cc_in_t = dram.tile([BL, 1], F32, tag="ccint")
            cc_out_t = dram.tile([B, 1], F32, tag="ccoutt", addr_space="Shared")
            nc.sync.dma_start(cc_in_t[:], lgt[:])
            nc.gpsimd.collective_compute(
                "AllGather", ALU.bypass,
                replica_groups=[list(range(NC))],
                ins=[cc_in_t[:].opt()],
                outs=[cc_out_t[:].opt()],
            )
            g_sb = small.tile([1, 2 * B], F32, tag="g")
            nc.sync.dma_start(g_sb[0:1, B:2 * B], cc_out_t[:, :])
            ht2_sb = small.tile([BL, H], F32, tag="ht2_sb")
            nc.scalar.copy(ht2_sb[:], ht2_ps[:])

            # ---------- frames: x-stream with per-pair softmax+einsum fused ----
            vbv_sb = load(wstream, vbv, (128, KT * H), "wstream", name="vbv_t", engine=nc.scalar)
            wveT_sb = load(wstream, wveT, (128, KT * H), "wstream", name="wveT_t", engine=nc.gpsimd)

            hvT_sb = small.tile([128, KT * BL], F32, tag="hvT")
            hvT_bf = small.tile([128, KT * BL], BF16, tag="hvT_bf")
            NP = BL // 2   # pairs
            yv_tiles = {}

            scv_tiles = {}
            scv_cnt = {}

            def scv_mm(g, jt):
                if g not in scv_tiles:
                    scv_tiles[g] = psS.tile([2, Tv], F32, tag="scS", name=f"scv{g}")
                    scv_cnt[g] = 0
                scv_g = scv_tiles[g]
                for i in range(2):
                    scv_cnt[g] += 1
                    nc.tensor.matmul(
                        scv_g[:],
                        vavZ_sb[:, jt * 4 + i * 2: jt * 4 + i * 2 + 2],
                        yv_tiles[(g, jt, i)][:],
                        start=(scv_cnt[g] == 1), stop=(scv_cnt[g] == 2 * KT),
                        skip_group_check=True,
                    )

            def pair_chain(g):
                """softmax -> broadcast -> einsum STT for pair g (scores done)."""
                bs = (2 * g, 2 * g + 1)
                scv_g = scv_tiles[g]
                expv = small.tile([2, Tv], F32, tag="expv", name=f"expv{g}", bufs=2)
                sumv = small.tile([2, 1], F32, tag="sumv", name=f"sumv{g}", bufs=2)
                nc.scalar.activation(
                    expv[:], scv_g[:], ACT.Exp, accum_out=sumv[:],
                )
                rv = small.tile([2, 1], F32, tag="rv", name=f"rv{g}", bufs=2)
                nc.vector.reciprocal(rv[:], sumv[:])
                avp = small.tile([2, Tv], BF16, tag="av", name=f"av{g}", bufs=2)
                nc.vector.tensor_scalar_mul(avp[:], expv[:], rv[:])
                avR = small.tile([1, 2 * Tv], BF16, tag="avR", name=f"avR{g}", bufs=2)
                nc.sync.dma_start(avR[0:1, :], avp[:, :])
                for i, b in enumerate(bs):
                    avB_ps = psB.tile([128, Tv], F32, tag="psB", name=f"avB{b}")
                    src = avp[0:1, :] if i == 0 else avR[0:1, Tv:2 * Tv]
                    nc.tensor.matmul(
                        avB_ps[:], ones_sb[0:1, 0:128], src,
                        start=True, stop=True,
                    )
                    avB = work.tile([128, Tv], BF16, tag="avB_sb", name=f"avBs{b}", bufs=4)
                    nc.scalar.copy(avB[:], avB_ps[:])
                    for kt in range(KT):
                        scr = work.tile([128, Tv], BF16, tag="scr")
                        nc.vector.scalar_tensor_tensor(
                            out=scr[:],
                            in0=fT_sb[b][:, kt * Tv:(kt + 1) * Tv],
                            scalar=1.0,
                            in1=avB[:],
                            op0=ALU.mult, op1=ALU.mult,
                            accum_out=hvT_sb[:, kt * BL + b: kt * BL + b + 1],
                        )

            for g in range(NP):
                bs = (2 * g, 2 * g + 1)
                for jt in range(KT):
                    xps = [psX.tile([128, Tv], F32, tag="psX", name=f"xps{g}_{jt}_{i}")
                           for i in range(2)]
                    for kt in range(KT):
                        for i, b in enumerate(bs):
                            nc.tensor.matmul(
                                xps[i][:],
                                wav_sb[:, kt * H + jt * 128: kt * H + (jt + 1) * 128],
                                fT_sb[b][:, kt * Tv:(kt + 1) * Tv],
                                start=(kt == 0), stop=(kt == KT - 1),
                            )
                    if jt >= 1:
                        scv_mm(g, jt - 1)          # one jt-stage behind
                    elif g >= 1:
                        scv_mm(g - 1, KT - 1)      # previous pair's last jt
                        pair_chain(g - 1)
                    for i, b in enumerate(bs):
                        yv = work.tile([128, Tv], BF16, tag="yv",
                                       name=f"yv{g}_{jt}_{i}", bufs=12)
                        nc.scalar.activation(
                            yv[:], xps[i][:], ACT.Tanh,
                            bias=uhvbT_sb[:, jt * BL + b: jt * BL + b + 1],
                        )
                        yv_tiles[(g, jt, i)] = yv
            scv_mm(NP - 1, KT - 1)
            pair_chain(NP - 1)

            # ---------- gates (mv1/hv2) once hvT complete ----------
            mv1_ps = psG.tile([BL, H], F32, tag="psG", name="mv1")
            gate_matmul(hT_sb, wbm_sb, bbv_r, ps=mv1_ps)   # h@Wb + bbv
            hv2_ps = psG.tile([BL, H], F32, tag="psG", name="hv2")
            nc.tensor.matmul(
                hv2_ps[:], ones_sb[0:1, 0:BL], bve_r,
                start=True, stop=False, skip_group_check=True,
            )
            for kt in range(KT):
                nc.vector.tensor_copy(
                    hvT_bf[:, kt * BL:(kt + 1) * BL],
                    hvT_sb[:, kt * BL:(kt + 1) * BL],
                )
                nc.tensor.matmul(
                    mv1_ps[:], hvT_bf[:, kt * BL:(kt + 1) * BL],
                    vbv_sb[:, kt * H:(kt + 1) * H],
                    start=False, stop=(kt == KT - 1), skip_group_check=True,
                )
                nc.tensor.matmul(
                    hv2_ps[:], hvT_bf[:, kt * BL:(kt + 1) * BL],
                    wveT_sb[:, kt * H:(kt + 1) * H],
                    start=False, stop=(kt == KT - 1), skip_group_check=True,
                )

            mtv_v = small.tile([BL, H], F32, tag="mtv_v")
            nc.scalar.activation(mtv_v[:], mv1_ps[:], ACT.Tanh)
            lgv = small.tile([BL, 1], F32, tag="lgv")
            scr8 = small.tile([BL, H], F32, tag="scr8")
            nc.vector.scalar_tensor_tensor(
                out=scr8[:], in0=mtv_v[:], scalar=1.0, in1=wbB_sb[:],
                op0=ALU.mult, op1=ALU.mult, accum_out=lgv[:],
            )

            # ---------- collective: AllGather the 8 visual logits ----------
            cc_in = dram.tile([BL, 1], F32, tag="ccin")
            cc_out = dram.tile([B, 1], F32, tag="ccout", addr_space="Shared")
            nc.sync.dma_start(cc_in[:], lgv[:])
            nc.gpsimd.collective_compute(
                "AllGather", ALU.bypass,
                replica_groups=[list(range(NC))],
                ins=[cc_in[:].opt()],
                outs=[cc_out[:].opt()],
            )

            # ---------- global beta softmax (no max-shift: logits are tiny) ----
            nc.sync.dma_start(g_sb[0:1, 0:B], cc_out[:, :])
            ge_sb = small.tile([1, 2 * B], F32, tag="ge")
            sumg = small.tile([1, 1], F32, tag="sumg")
            nc.scalar.activation(ge_sb[:], g_sb[:], ACT.Exp, accum_out=sumg[:])
            rg = small.tile([1, 1], F32, tag="rg")
            nc.vector.reciprocal(rg[:], sumg[:])
            betas = small.tile([1, 2], BF16, tag="betas")
            nc.vector.tensor_scalar_mul(betas[:], ge_sb[0:1, 0:2], rg[:])
            beta8_ps = psB.tile([BL, 2], F32, tag="psB", name="beta8")
            nc.tensor.matmul(
                beta8_ps[:], ones_sb[0:1, 0:BL], betas[0:1, 0:2],
                start=True, stop=True,
            )
            # ---------- out = tanh(hWhh+bh + b0*hv2 + b1*ht2) ----------
            t1 = small.tile([BL, H], F32, tag="t1")
            nc.vector.scalar_tensor_tensor(
                out=t1[:], in0=hv2_ps[:], scalar=beta8_ps[:, 0:1], in1=hwhh_sb[:],
                op0=ALU.mult, op1=ALU.add,
            )
            s1 = small.tile([BL, H], F32, tag="s1")
            nc.vector.scalar_tensor_tensor(
                out=s1[:], in0=ht2_sb[:], scalar=beta8_ps[:, 1:2], in1=t1[:],
                op0=ALU.mult, op1=ALU.add,
            )
            out_sb = small.tile([BL, H], F32, tag="out_sb")
            nc.scalar.activation(out_sb[:], s1[:], ACT.Tanh)
            nc.sync.dma_start(out_ext, out_sb[:])

    nc.compile()
    return nc


_cached_nc = None


def _get_nc():
    global _cached_nc
    if _cached_nc is None:
        _cached_nc = build_nc()
    return _cached_nc


def _bf(a):
    return np.asarray(a, np.float32).astype(NP_BF16)


def _pack_w(w):
    """[512,512] -> [128, 4*512] with free = kt*512 + j, partition p: k=kt*128+p."""
    return np.ascontiguousarray(
        np.asarray(w, np.float32).reshape(KT, 128, H).transpose(1, 0, 2)
        .reshape(128, KT * H)
    ).astype(NP_BF16)


def make_in_maps(inputs):
    h = np.asarray(inputs["h"], np.float32)
    frames = np.asarray(inputs["hidden_frames"], np.float32)
    text = np.asarray(inputs["hidden_text"], np.float32)

    Vav = np.asarray(inputs["Vav"], np.float32)
    Vat = np.asarray(inputs["Vat"], np.float32)
    wb = np.asarray(inputs["wb"], np.float32)

    vavZ = np.zeros((128, KT, 2, 2), np.float32)
    for jt in range(KT):
        for i in range(2):
            vavZ[:, jt, i, i] = Vav[jt * 128:(jt + 1) * 128]
    vavZ = _bf(vavZ.reshape(128, KT * 4))
    vatT = _bf(np.ascontiguousarray(Vat.reshape(KT, 128).T))

    biasr = np.zeros((1, 8 * H), np.float32)
    for i, k in enumerate(["bav", "bat", "bve", "bqe", "bbv", "bbt", "bh"]):
        biasr[0, i * H:(i + 1) * H] = np.asarray(inputs[k], np.float32)
    biasr = _bf(biasr)
    wbB = np.ascontiguousarray(np.broadcast_to(wb, (BL, H))).astype(np.float32)
    eye = _bf(np.eye(128, dtype=np.float32))
    blkI = np.zeros((BL, BL, Tt), np.float32)
    for b in range(BL):
        blkI[b, b, :] = 1.0
    blkI = _bf(blkI.reshape(BL, BL * Tt))

    shared = dict(
        wav=_pack_w(inputs["Wav"]), wat=_pack_w(inputs["Wat"]),
        uav=_pack_w(inputs["Uav"]), uat=_pack_w(inputs["Uat"]),
        whh=_pack_w(inputs["Whh"]), wb_m=_pack_w(inputs["Wb"]),
        wveT=_pack_w(np.asarray(inputs["Wve"], np.float32).T),
        wqeT=_pack_w(np.asarray(inputs["Wqe"], np.float32).T),
        vbv=_pack_w(inputs["Vbv"]), vbt=_pack_w(inputs["Vbt"]),
        vavZ=vavZ, vatT=vatT, biasr=biasr, wbB=wbB, eye=eye, blkI=blkI,
    )

    in_maps = []
    for i in range(NC):
        sl = slice(i * BL, (i + 1) * BL)
        fTc = np.ascontiguousarray(
            frames[sl].transpose(0, 2, 1)       # [BL, H, Tv]
            .reshape(BL, KT, 128, Tv)
            .transpose(0, 2, 1, 3)              # [BL, 128, KT, Tv]
            .reshape(BL, 128, KT * Tv)
        ).astype(NP_BF16)
        tTc = np.ascontiguousarray(
            text[sl].transpose(2, 0, 1)         # [H, BL, Tt]
            .reshape(KT, 128, BL, Tt)
            .transpose(1, 0, 2, 3)              # [128, KT, BL, Tt]
            .reshape(128, KT * BL * Tt)
        ).astype(NP_BF16)
        hTc = _bf(
            h[sl].T.reshape(KT, 128, BL).transpose(1, 0, 2).reshape(128, KT * BL)
        )
        in_maps.append(dict(shared, fT=fTc, tT=tTc, hT=hTc))
    return in_maps


def run(inputs, trace=False, **kw):
    nc = _get_nc()
    in_maps = make_in_maps(inputs)
    res = run_bass_kernel_spmd(nc, in_maps, core_ids=list(range(NC)), trace=trace, **kw)
    out = np.concatenate([res.results[i]["out"] for i in range(NC)], axis=0)
    return out, res


def kernel(**inputs) -> np.ndarray:
    out, _ = run(inputs, trace=False)
    return out



# revision 2
# speedup vs baseline: 2.1314x; 2.1314x over previous
"""Trainium2 Bass kernel v3 for nn_AttentionTwoStream — fp8 DoubleRow edition.

Sharding: data-parallel over batch B=64 -> 8 batches/core; weights replicated.
Cross-core coupling: beta softmax over 2B logits -> ONE 16-float AllGather
(first-collective ncfw init dominates; a single collective minimizes the
serialized post-init chain).

Compute strategy:
- Big matmuls (frames/text/h-projections) in fp8e4 DoubleRow (2 k-subtiles
  per instruction). Weights pre-scaled x64 on host (fp8e4 min-normal is
  2^-6; raw 0.01-scale weights would be subnormal); the x64 is divided out
  in the downstream activation's `scale`.
- Whh / Vb* / Wqe / Wve paths stay bf16 (h@Whh dominates the output).
- tanh/exp on ACT; all PSUM evacuations + einsums on DVE/GpSimd.
- av/at partition-broadcasts via DRAM round-trip DMA (zero engine time).

Self-contained: hardcodes B=64, Tv=512, Tt=64, H=512, 8 cores.
"""

import numpy as np
import ml_dtypes

import concourse.bacc as bacc
import concourse.bass as bass
import concourse.mybir as mybir
import concourse.tile as tile
from concourse.bass_utils import run_bass_kernel_spmd

NC = 8
B = 64
BL = B // NC    # 8
H = 512
Tv = 512
Tt = 64
KT = H // 128   # 4
WS = 64.0       # fp8 weight pre-scale
F32 = mybir.dt.float32
BF16 = mybir.dt.bfloat16
FP8 = mybir.dt.float8e4
NP_BF16 = ml_dtypes.bfloat16
NP_FP8 = ml_dtypes.float8_e4m3
DR = mybir.MatmulPerfMode.DoubleRow


def build_nc():
    nc = bacc.Bacc(
        "TRN2", target_bir_lowering=False, debug=False,
        enable_asserts=False, num_devices=NC, num_swdge_queues=4,
    )

    def inp(name, shape, dt):
        return nc.dram_tensor(name, list(shape), dt, kind="ExternalInput").ap()

    # --- external inputs (per-core shards; layouts match SBUF tiles) ---
    fT = inp("fT", (BL, 128, KT, Tv), FP8)     # frames^T [b][k%128][kt][t]
    tT = inp("tT", (128, KT, BL * Tt), FP8)    # text^T
    hT8 = inp("hT8", (128, KT, 16), FP8)   # b-dim padded to 16 (DR stride align)       # h^T fp8
    hT16 = inp("hT16", (128, KT * BL), BF16)   # h^T bf16 (Whh path)
    wavS = inp("wavS", (128, KT, H), FP8)      # 64*Wav
    watS = inp("watS", (128, KT, H), FP8)
    uavS = inp("uavS", (128, KT, H), FP8)
    uatS = inp("uatS", (128, KT, H), FP8)
    wb8 = inp("wb8", (128, KT, H), FP8)        # 64*Wb
    whh = inp("whh", (128, KT * H), BF16)
    wveT8 = inp("wveT8", (128, KT, H), FP8)    # 64*Wve.T
    wqeT8 = inp("wqeT8", (128, KT, H), FP8)
    vbv8 = inp("vbv8", (128, KT, H), FP8)
    vbt8 = inp("vbt8", (128, KT, H), FP8)
    eyeD = inp("eyeD", (BL, BL), BF16)         # 16*I (wbs scale injection)
    vavZ = inp("vavZ", (128, KT, 16), FP8)      # [p,jt,i*2+m]: m==i -> 64*Vav
    vatZ = inp("vatZ", (128, KT, 16), FP8)      # 64*Vat
    biasr = inp("biasr", (1, 8 * H), BF16)     # 64*bav,64*bat,bve,bqe,bbv,bbt,bh
    wbB = inp("wbB", (BL, H), F32)             # wb broadcast to 8 partitions
    eye = inp("eye", (128, 128), BF16)
    blkI = inp("blkI", (BL, BL * Tt), BF16)    # blkI[b, b'*64+t] = (b == b')

    out_ext = nc.dram_tensor("out", [BL, H], F32, kind="ExternalOutput").ap()

    ACT = mybir.ActivationFunctionType
    ALU = mybir.AluOpType
    R64 = 1.0 / WS

    with tile.TileContext(nc) as tc:
        with (
            tc.tile_pool(name="wres", bufs=1) as wres,
            tc.tile_pool(name="work", bufs=2) as work,
            tc.tile_pool(name="small", bufs=1) as small,
            tc.tile_pool(name="psX", bufs=3, space="PSUM") as psX,
            tc.tile_pool(name="psS", bufs=2, space="PSUM") as psS,
            tc.tile_pool(name="psB", bufs=1, space="PSUM") as psB,
            tc.tile_pool(name="psG", bufs=2, space="PSUM") as psG,
            tc.tile_pool(name="dram", bufs=1, space="DRAM") as dram,
        ):
            # ---------- warmup collective at t=0 (absorbs ncfw init + skew) --
            warm_out = dram.tile([2 * B, 1], F32, tag="warmout", addr_space="Shared")
            warm_in = dram.tile([2 * BL, 1], F32, tag="warmin")
            nc.gpsimd.collective_compute(
                "AllGather", ALU.bypass,
                replica_groups=[list(range(NC))],
                ins=[warm_in[:].opt()],
                outs=[warm_out[:].opt()],
            )

            # ---------- PE warmup on a memset tile (no DMA dependency) -------
            warmA = small.tile([128, 128], BF16, tag="warmA")
            nc.vector.memset(warmA[:], 0.001)
            warm_ps = psB.tile([128, 128], F32, tag="psB", name="warmps")
            for _ in range(28):
                nc.tensor.matmul(
                    warm_ps[:], warmA[:], warmA[:],
                    start=True, stop=True, skip_group_check=True,
                )

            # ---------- DMAs (priority order) --------------------------------
            def load(pool, ap_in, shape, dt, tag, engine, name=None):
                t = pool.tile(list(shape), dt, tag=tag, name=name or tag)
                engine.dma_start(t[:], ap_in)
                return t

            fT_sb = [None] * BL
            def load_fT(b, eng):
                t = wres.tile([128, KT, Tv], FP8, tag=f"fT{b}", name=f"fTs{b}")
                eng.dma_start(t[:], fT[b])
                fT_sb[b] = t
            # sync queue: P1 controls then fT0/fT1 (fastest start)
            hT8_sb = load(wres, hT8, (128, KT, 16), FP8, "hT8", nc.sync)
            biasr_sb = load(wres, biasr, (1, 8 * H), BF16, "biasr", nc.sync)
            eye_sb = load(wres, eye, (128, 128), BF16, "eye", nc.sync)
            vavZ_sb = load(wres, vavZ, (128, KT, 16), FP8, "vavZ", nc.sync)
            load_fT(0, nc.sync)
            load_fT(1, nc.sync)
            hT16_sb = load(wres, hT16, (128, KT * BL), BF16, "hT16", nc.sync)
            wbB_sb = load(wres, wbB, (BL, H), F32, "wbB", nc.sync)
            blkI_sb = load(wres, blkI, (BL, BL * Tt), BF16, "blkI", nc.sync)
            vatZ_sb = load(wres, vatZ, (128, KT, 16), FP8, "vatZ", nc.sync)
            eyeD_sb = load(wres, eyeD, (BL, BL), BF16, "eyeD", nc.sync)
            wb8_sb = load(wres, wb8, (128, KT, H), FP8, "wb8", nc.sync)
            vbt8_sb = load(wres, vbt8, (128, KT, H), FP8, "vbt8", nc.sync)
            wqeT8_sb = load(wres, wqeT8, (128, KT, H), FP8, "wqeT8", nc.sync)
            # scalar queue: P1 + frames weights first, late weights after
            uavS_sb = load(wres, uavS, (128, KT, H), FP8, "uavS", nc.scalar)
            wavS_sb = load(wres, wavS, (128, KT, H), FP8, "wavS", nc.scalar)
            uatS_sb = load(wres, uatS, (128, KT, H), FP8, "uatS", nc.scalar)
            watS_sb = load(wres, watS, (128, KT, H), FP8, "watS", nc.scalar)
            whh_sb = load(wres, whh, (128, KT * H), BF16, "whh", nc.scalar)
            vbv8_sb = load(wres, vbv8, (128, KT, H), FP8, "vbv8", nc.scalar)
            wveT8_sb = load(wres, wveT8, (128, KT, H), FP8, "wveT8", nc.scalar)
            # gpsimd: remaining frames + text
            load_fT(2, nc.gpsimd)
            load_fT(3, nc.gpsimd)
            tT_sb = load(wres, tT, (128, KT, BL * Tt), FP8, "tT", nc.gpsimd)
            for b in range(4, BL):
                load_fT(b, nc.gpsimd)

            ones_sb = small.tile([1, 128], BF16, tag="ones")
            nc.vector.memset(ones_sb[:], 1.0)

            def brow(i):
                return biasr_sb[0:1, i * H:(i + 1) * H]
            bavS_r, batS_r, bve_r, bqe_r, bbv_r, bbt_r, bh_r = (brow(i) for i in range(7))

            # fp8 DoubleRow gate: out[8,512] = 64*(h@W + bias);  2 DR + 1 bias mm
            def gate8(w_sb, bias_row, name):
                ps = psG.tile([BL, H], F32, tag="psG", name=name)
                for q in range(2):
                    nc.tensor.matmul(
                        ps[:], hT8_sb[:, 2 * q:2 * q + 2, 0:BL],
                        w_sb[:, 2 * q:2 * q + 2, :],
                        start=(q == 0), stop=False, perf_mode=DR,
                        skip_group_check=True,
                    )
                nc.tensor.matmul(
                    ps[:], ones_sb[0:1, 0:BL], bias_row,
                    start=False, stop=True, skip_group_check=True,
                )
                return ps

            # bf16 gate: accum into ps; lhsT [128, 8] kt-sliced
            def gate16(lhsT_sb, w_sb, bias_row, ps=None, name="g16",
                       start=True, stop=True):
                if ps is None:
                    ps = psG.tile([BL, H], F32, tag="psG", name=name)
                for kt in range(KT):
                    nc.tensor.matmul(
                        ps[:], lhsT_sb[:, kt * BL:(kt + 1) * BL],
                        w_sb[:, kt * H:(kt + 1) * H],
                        start=(start and kt == 0), stop=False,
                        skip_group_check=True,
                    )
                nc.tensor.matmul(
                    ps[:], ones_sb[0:1, 0:BL], bias_row,
                    start=False, stop=stop, skip_group_check=True,
                )
                return ps

            # ---------- P1: h projections (fp8, x64-scaled) ------------------
            uhvb_ps = gate8(uavS_sb, bavS_r, "uhvb")     # 64*(Uhv + bav)
            uhtb_ps = gate8(uatS_sb, batS_r, "uhtb")

            uhvb_s = small.tile([BL, H], BF16, tag="uhvb_s")   # scaled
            nc.vector.tensor_copy(uhvb_s[:], uhvb_ps[:])
            uhtb_s = small.tile([BL, H], BF16, tag="uhtb_s")   # scaled
            nc.vector.tensor_copy(uhtb_s[:], uhtb_ps[:])

            # uhvbT: [j, (jt,b)] UNSCALED f32 for ACT bias
            uhvbT_sb = small.tile([128, KT * BL], F32, tag="uhvbT")
            for jt in range(KT):
                tp = psB.tile([128, BL], BF16, tag="psB", name=f"tpv{jt}")
                nc.tensor.transpose(
                    tp[:], uhvb_s[0:BL, jt * 128:(jt + 1) * 128],
                    eye_sb[0:BL, 0:BL],
                )
                nc.vector.tensor_scalar_mul(
                    uhvbT_sb[:, jt * BL:(jt + 1) * BL], tp[:], R64,
                )

            # ---------- frames stream state ----------
            hvT_sb = small.tile([128, KT * BL], F32, tag="hvT")
            NP_ = BL // 2
            yv_tiles = {}
            scv_tiles = {}

            def frames_stage(g, jt):
                """Emit main DR matmuls for (pair g, out-tile jt)."""
                bs = (2 * g, 2 * g + 1)
                xps = [psX.tile([128, Tv], F32, tag="psX", name=f"xp{g}_{jt}_{i}")
                       for i in range(2)]
                for q in range(2):
                    for i, b in enumerate(bs):
                        nc.tensor.matmul(
                            xps[i][:],
                            wavS_sb[:, 2 * q:2 * q + 2, jt * 128:(jt + 1) * 128],
                            fT_sb[b][:, 2 * q:2 * q + 2, :],
                            start=(q == 0), stop=(q == 1), perf_mode=DR,
                        )
                return xps

            def frames_tanh(g, jt, xps):
                if (g, 0) not in yv_tiles:
                    for qq in range(2):
                        yv_tiles[(g, qq)] = work.tile(
                            [128, 2, 2, Tv], FP8, tag="yv4", name=f"yv4_{g}_{qq}",
                            bufs=4,
                        )
                q, jj = jt // 2, jt % 2
                for i, b in enumerate((2 * g, 2 * g + 1)):
                    nc.scalar.activation(
                        yv_tiles[(g, q)][:, jj, i, :], xps[i][:], ACT.Tanh,
                        bias=uhvbT_sb[:, jt * BL + b: jt * BL + b + 1],
                        scale=R64,
                    )

            def scv_mm(g, q):
                """Score DR matmuls for jt-pair q of pair g."""
                if g not in scv_tiles:
                    scv_tiles[g] = psS.tile([2, Tv], F32, tag="psS", name=f"scv{g}")
                scv_g = scv_tiles[g]
                yv4 = yv_tiles[(g, q)]
                for i in range(2):
                    nc.tensor.matmul(
                        scv_g[:], vavZ_sb[:, 2 * q:2 * q + 2, 2 * i:2 * i + 2],
                        yv4[:, :, i, :],
                        start=(q == 0 and i == 0), stop=(q == 1 and i == 1),
                        perf_mode=DR, skip_group_check=True,
                    )

            def pair_chain(g):
                """softmax -> DRAM-broadcast -> einsum for pair g."""
                bs = (2 * g, 2 * g + 1)
                scv_g = scv_tiles[g]
                expv = small.tile([2, Tv], F32, tag="expv", name=f"expv{g}", bufs=2)
                sumv = small.tile([2, 1], F32, tag="sumv", name=f"sumv{g}", bufs=2)
                nc.scalar.activation(
                    expv[:], scv_g[:], ACT.Exp, scale=R64, accum_out=sumv[:],
                )
                rv = small.tile([2, 1], F32, tag="rv", name=f"rv{g}", bufs=2)
                nc.vector.reciprocal(rv[:], sumv[:])
                avp = small.tile([2, Tv], BF16, tag="av", name=f"av{g}", bufs=2)
                nc.vector.tensor_scalar_mul(avp[:], expv[:], rv[:])
                avR = small.tile([1, 2 * Tv], BF16, tag="avR", name=f"avR{g}", bufs=2)
                nc.sync.dma_start(avR[0:1, :], avp[:, :])
                scr = work.tile([128, Tv], BF16, tag="scr", name=f"scr{g}", bufs=2)
                for i, b in enumerate(bs):
                    avB_ps = psB.tile([128, Tv], F32, tag="psB", name=f"avB{b}")
                    bsrc = avp[0:1, :] if i == 0 else avR[0:1, Tv:2 * Tv]
                    nc.tensor.matmul(
                        avB_ps[:], ones_sb[0:1, 0:128], bsrc,
                        start=True, stop=True, skip_group_check=True,
                    )
                    avB = work.tile([128, Tv], BF16, tag="avB", name=f"avB{g}_{i}",
                                    bufs=4)
                    nc.vector.tensor_copy(avB[:], avB_ps[:])
                    for kt in range(KT):
                        nc.vector.scalar_tensor_tensor(
                            out=scr[:],
                            in0=fT_sb[b][:, kt, :],
                            scalar=1.0,
                            in1=avB[:],
                            op0=ALU.mult, op1=ALU.mult,
                            accum_out=hvT_sb[:, kt * BL + b: kt * BL + b + 1],
                        )

            # ---------- frames pairs 0-1 (earliest DMA arrivals) ----------
            for g in range(2):
                for jt in range(KT):
                    xps = frames_stage(g, jt)
                    if g == 1 and jt == 0:
                        scv_mm(0, 0)
                    elif g == 1 and jt == 2:
                        scv_mm(0, 1)
                        pair_chain(0)
                    frames_tanh(g, jt, xps)

            # ---------- text stream (fp8 DR): Xt' = 64*(Wat.T@tT + Uht + bat)
            yt4 = small.tile([128, KT, BL * Tt], FP8, tag="yt4")
            for jt in range(KT):
                xt_ps = psX.tile([128, BL * Tt], F32, tag="psX", name=f"xt{jt}")
                for q in range(2):
                    nc.tensor.matmul(
                        xt_ps[:],
                        watS_sb[:, 2 * q:2 * q + 2, jt * 128:(jt + 1) * 128],
                        tT_sb[:, 2 * q:2 * q + 2, :],
                        start=(q == 0), stop=False, perf_mode=DR,
                        skip_group_check=True,
                    )
                nc.tensor.matmul(      # + 64*(Uht+bat) via block identity
                    xt_ps[:], uhtb_s[0:BL, jt * 128:(jt + 1) * 128], blkI_sb[:],
                    start=False, stop=True, skip_group_check=True,
                )
                nc.scalar.activation(yt4[:, jt, :], xt_ps[:], ACT.Tanh, scale=R64)

            scv_mm(1, 0)

            # hWhh + bh (bf16)
            hwhh_ps = gate16(hT16_sb, whh_sb, bh_r, name="hwhh")
            hwhh_sb = small.tile([BL, H], F32, tag="hwhh_sb")
            nc.vector.tensor_copy(hwhh_sb[:], hwhh_ps[:])

            # sct: scores_t = 64 * yt @ Vat   (fp8 DR)
            sct_ps = psS.tile([1, BL * Tt], F32, tag="psS", name="sct")
            for q in range(2):
                nc.tensor.matmul(
                    sct_ps[:], vatZ_sb[:, 2 * q:2 * q + 2, 0:1],
                    yt4[:, 2 * q:2 * q + 2, :],
                    start=(q == 0), stop=(q == 1), perf_mode=DR,
                    skip_group_check=True,
                )

            # ---------- text softmax ----------
            sct_sb = small.tile([1, BL * Tt], F32, tag="sct_sb")
            nc.vector.tensor_copy(sct_sb[:], sct_ps[:])
            st8 = small.tile([BL, Tt], F32, tag="st8")
            nc.sync.dma_start(st8[:, :], sct_sb[0:1, :])
            expt_sb = small.tile([BL, Tt], F32, tag="expt")
            sumt = small.tile([BL, 1], F32, tag="sumt")
            nc.scalar.activation(
                expt_sb[:], st8[:], ACT.Exp, scale=R64, accum_out=sumt[:],
            )
            rt = small.tile([BL, 1], F32, tag="rt")
            nc.vector.reciprocal(rt[:], sumt[:])
            at_sb = small.tile([BL, Tt], BF16, tag="at")
            nc.vector.tensor_scalar_mul(at_sb[:], expt_sb[:], rt[:])

            # atB: broadcast at across partitions via PE ones-matmul
            atRows = small.tile([1, BL * Tt], BF16, tag="atRows")
            nc.sync.dma_start(atRows[0:1, :], at_sb[:, :])
            atB_ps = psB.tile([128, BL * Tt], F32, tag="psB", name="atB")
            for b in range(BL):
                bsrc = at_sb[0:1, :] if b == 0 else atRows[0:1, b * Tt:(b + 1) * Tt]
                nc.tensor.matmul(
                    atB_ps[:, b * Tt:(b + 1) * Tt], ones_sb[0:1, 0:128], bsrc,
                    start=True, stop=True, skip_group_check=True,
                )
            atB_sb = small.tile([128, BL * Tt], BF16, tag="atB_sb")
            nc.vector.tensor_copy(atB_sb[:], atB_ps[:])

            # text einsum on DVE: htT[:, kt*8+b] = sum_t tT*at
            htT_sb = small.tile([128, KT * BL], F32, tag="htT")
            scrt = small.tile([128, Tt], BF16, tag="scrt")
            for kt in range(KT):
                for b in range(BL):
                    nc.vector.scalar_tensor_tensor(
                        out=scrt[:],
                        in0=tT_sb[:, kt, b * Tt:(b + 1) * Tt],
                        scalar=1.0,
                        in1=atB_sb[:, b * Tt:(b + 1) * Tt],
                        op0=ALU.mult, op1=ALU.mult,
                        accum_out=htT_sb[:, kt * BL + b: kt * BL + b + 1],
                    )

            # ---------- text gates + lgt into cc_in (all x1024-scaled) -------
            # wbs = 64*(h@Wb)  (shared by mt1 and mv1)
            wbs_ps = psG.tile([BL, H], F32, tag="psG", name="wbs")
            for q in range(2):
                nc.tensor.matmul(
                    wbs_ps[:], hT8_sb[:, 2 * q:2 * q + 2, 0:BL],
                    wb8_sb[:, 2 * q:2 * q + 2, :],
                    start=(q == 0), stop=(q == 1), perf_mode=DR,
                    skip_group_check=True,
                )
            wbs_sb = small.tile([BL, H], BF16, tag="wbs_sb")
            nc.vector.tensor_copy(wbs_sb[:], wbs_ps[:])

            def gate1024(lhsT8, w8, bias_row, name):
                """x1024 group: 16*eyeD@wbs(64x) + bias(1024x) + lhsT8@w8."""
                ps = psG.tile([BL, H], F32, tag="psG", name=name)
                nc.tensor.matmul(
                    ps[:], eyeD_sb[:], wbs_sb[:],
                    start=True, stop=False, skip_group_check=True,
                )
                nc.tensor.matmul(
                    ps[:], ones_sb[0:1, 0:BL], bias_row,
                    start=False, stop=False, skip_group_check=True,
                )
                for q in range(2):
                    nc.tensor.matmul(
                        ps[:], lhsT8[:, 2 * q:2 * q + 2, 0:BL],
                        w8[:, 2 * q:2 * q + 2, :],
                        start=False, stop=(q == 1), perf_mode=DR,
                        skip_group_check=True,
                    )
                return ps

            def gate1024_nb(lhsT8, w8, bias_row, name):
                """x1024 group without wbs: bias(1024x) + lhsT8@w8."""
                ps = psG.tile([BL, H], F32, tag="psG", name=name)
                nc.tensor.matmul(
                    ps[:], ones_sb[0:1, 0:BL], bias_row,
                    start=True, stop=False, skip_group_check=True,
                )
                for q in range(2):
                    nc.tensor.matmul(
                        ps[:], lhsT8[:, 2 * q:2 * q + 2, 0:BL],
                        w8[:, 2 * q:2 * q + 2, :],
                        start=False, stop=(q == 1), perf_mode=DR,
                        skip_group_check=True,
                    )
                return ps

            # htT8: 16*ht_sum in fp8, padded [128, KT, 16]
            htT8 = small.tile([128, KT, 16], FP8, tag="htT8")
            for kt in range(KT):
                nc.vector.tensor_scalar_mul(
                    htT8[:, kt, 0:BL], htT_sb[:, kt * BL:(kt + 1) * BL], 16.0,
                )
            mt1_ps = gate1024(htT8, vbt8_sb, bbt_r, "mt1")
            ht2_ps = gate1024_nb(htT8, wqeT8_sb, bqe_r, "ht2")

            scv_mm(1, 1)
            pair_chain(1)

            lgt = small.tile([BL, 1], F32, tag="lgt")
            g_sb = small.tile([1, 2 * B], F32, tag="g")
            cc_in = dram.tile([2 * BL, 1], F32, tag="ccin")
            cc_out = dram.tile([2 * B, 1], F32, tag="ccout", addr_space="Shared")

            mtv_t = small.tile([BL, H], F32, tag="mtv_t")
            nc.scalar.activation(mtv_t[:], mt1_ps[:], ACT.Tanh, scale=1.0 / 1024.0)
            scr8b = small.tile([BL, H], F32, tag="scr8b")
            nc.vector.scalar_tensor_tensor(
                out=scr8b[:], in0=mtv_t[:], scalar=1.0, in1=wbB_sb[:],
                op0=ALU.mult, op1=ALU.mult, accum_out=lgt[:],
            )
            nc.sync.dma_start(cc_in[BL:2 * BL, :], lgt[:])
            # evacuate ht2 early (frees its PSUM bank before mv1/hv2)
            ht2_sb = small.tile([BL, H], F32, tag="ht2_sb")
            nc.vector.tensor_scalar_mul(ht2_sb[:], ht2_ps[:], 1.0 / 1024.0)

            # ---------- frames pairs 2-3 ----------
            for g in range(2, NP_):
                for jt in range(KT):
                    xps = frames_stage(g, jt)
                    if g == 3 and jt == 0:
                        scv_mm(2, 0)
                    elif g == 3 and jt == 2:
                        scv_mm(2, 1)
                        pair_chain(2)
                    frames_tanh(g, jt, xps)

            scv_mm(NP_ - 1, 0)
            scv_mm(NP_ - 1, 1)
            pair_chain(NP_ - 1)

            # ---------- visual gates (fp8, x1024) ----------
            hvT8 = small.tile([128, KT, 16], FP8, tag="hvT8")
            for kt in range(KT):
                nc.vector.tensor_scalar_mul(
                    hvT8[:, kt, 0:BL], hvT_sb[:, kt * BL:(kt + 1) * BL], 16.0,
                )
            mv1_ps = gate1024(hvT8, vbv8_sb, bbv_r, "mv1")
            hv2_ps = gate1024_nb(hvT8, wveT8_sb, bve_r, "hv2")

            mtv_v = small.tile([BL, H], F32, tag="mtv_v")
            nc.scalar.activation(mtv_v[:], mv1_ps[:], ACT.Tanh, scale=1.0 / 1024.0)
            lgv = small.tile([BL, 1], F32, tag="lgv")
            scr8 = small.tile([BL, H], F32, tag="scr8")
            nc.vector.scalar_tensor_tensor(
                out=scr8[:], in0=mtv_v[:], scalar=1.0, in1=wbB_sb[:],
                op0=ALU.mult, op1=ALU.mult, accum_out=lgv[:],
            )

            nc.sync.dma_start(cc_in[0:BL, :], lgv[:])
            nc.gpsimd.collective_compute(
                "AllGather", ALU.bypass,
                replica_groups=[list(range(NC))],
                ins=[cc_in[:].opt()],
                outs=[cc_out[:].opt()],
            )

            # ---------- global beta softmax + output ----------
            # g_sb layout: [core0: lgv(8), lgt(8); core1: ...] — order is
            # irrelevant for the softmax sum; beta0/1 = exp(g[0:2])/sum.
            nc.sync.dma_start(g_sb[0:1, :], cc_out[:, :])
            ge_sb = small.tile([1, 2 * B], F32, tag="ge")
            sumg = small.tile([1, 1], F32, tag="sumg")
            nc.scalar.activation(ge_sb[:], g_sb[:], ACT.Exp, accum_out=sumg[:])
            rg = small.tile([1, 1], F32, tag="rg")
            nc.vector.reciprocal(rg[:], sumg[:])
            betas = small.tile([1, 2], BF16, tag="betas")
            nc.vector.tensor_scalar_mul(betas[:], ge_sb[0:1, 0:2], rg[:])
            beta8_ps = psB.tile([BL, 2], F32, tag="psB", name="beta8")
            nc.tensor.matmul(
                beta8_ps[:], ones_sb[0:1, 0:BL], betas[0:1, 0:2],
                start=True, stop=True, skip_group_check=True,
            )
            hv2_sb = small.tile([BL, H], F32, tag="hv2_sb")
            nc.vector.tensor_scalar_mul(hv2_sb[:], hv2_ps[:], 1.0 / 1024.0)
            t1 = small.tile([BL, H], F32, tag="t1")
            nc.vector.scalar_tensor_tensor(
                out=t1[:], in0=hv2_sb[:], scalar=beta8_ps[:, 0:1], in1=hwhh_sb[:],
                op0=ALU.mult, op1=ALU.add,
            )
            s1 = small.tile([BL, H], F32, tag="s1")
            nc.vector.scalar_tensor_tensor(
                out=s1[:], in0=ht2_sb[:], scalar=beta8_ps[:, 1:2], in1=t1[:],
                op0=ALU.mult, op1=ALU.add,
            )
            out_sb = small.tile([BL, H], F32, tag="out_sb")
            nc.scalar.activation(out_sb[:], s1[:], ACT.Tanh)
            nc.sync.dma_start(out_ext, out_sb[:])

    nc.compile()
    return nc


_cached_nc = None


def _get_nc():
    global _cached_nc
    if _cached_nc is None:
        _cached_nc = build_nc()
    return _cached_nc


def _pack_w8(w, scale=WS):
    """[512,512] -> [128, KT, H] fp8, [p, kt, j] = scale*w[kt*128+p, j]."""
    a = np.asarray(w, np.float32).reshape(KT, 128, H).transpose(1, 0, 2) * scale
    return np.ascontiguousarray(a).astype(NP_FP8)


def _pack_w16(w):
    a = np.asarray(w, np.float32).reshape(KT, 128, H).transpose(1, 0, 2)
    return np.ascontiguousarray(a.reshape(128, KT * H)).astype(NP_BF16)


def make_in_maps(inputs):
    h = np.asarray(inputs["h"], np.float32)
    frames = np.asarray(inputs["hidden_frames"], np.float32)
    text = np.asarray(inputs["hidden_text"], np.float32)
    Vav = np.asarray(inputs["Vav"], np.float32)
    Vat = np.asarray(inputs["Vat"], np.float32)
    wb = np.asarray(inputs["wb"], np.float32)

    vavZ = np.zeros((128, KT, 16), np.float32)
    for jt in range(KT):
        for i in range(2):
            vavZ[:, jt, 2 * i + i] = WS * Vav[jt * 128:(jt + 1) * 128]
    vavZ = vavZ.astype(NP_FP8)
    vatZ = np.zeros((128, KT, 16), np.float32)
    vatZ[:, :, 0] = (WS * Vat).reshape(KT, 128).T
    vatZ = vatZ.astype(NP_FP8)

    biasr = np.zeros((1, 8 * H), np.float32)
    for i, (k, sc) in enumerate([
        ("bav", WS), ("bat", WS), ("bve", 1024.0), ("bqe", 1024.0),
        ("bbv", 1024.0), ("bbt", 1024.0), ("bh", 1.0),
    ]):
        biasr[0, i * H:(i + 1) * H] = sc * np.asarray(inputs[k], np.float32)
    biasr = biasr.astype(NP_BF16)
    wbB = np.ascontiguousarray(np.broadcast_to(wb, (BL, H))).astype(np.float32)
    eye = np.eye(128, dtype=np.float32).astype(NP_BF16)
    blkI = np.zeros((BL, BL, Tt), np.float32)
    for b in range(BL):
        blkI[b, b, :] = 1.0
    blkI = blkI.reshape(BL, BL * Tt).astype(NP_BF16)

    eyeD = (16.0 * np.eye(BL, dtype=np.float32)).astype(NP_BF16)
    shared = dict(
        wavS=_pack_w8(inputs["Wav"]), watS=_pack_w8(inputs["Wat"]),
        uavS=_pack_w8(inputs["Uav"]), uatS=_pack_w8(inputs["Uat"]),
        wb8=_pack_w8(inputs["Wb"]), whh=_pack_w16(inputs["Whh"]),
        wveT8=_pack_w8(np.asarray(inputs["Wve"], np.float32).T),
        wqeT8=_pack_w8(np.asarray(inputs["Wqe"], np.float32).T),
        vbv8=_pack_w8(inputs["Vbv"]), vbt8=_pack_w8(inputs["Vbt"]),
        vavZ=vavZ, vatZ=vatZ, biasr=biasr, wbB=wbB, eye=eye, blkI=blkI,
        eyeD=eyeD,
    )

    in_maps = []
    for i in range(NC):
        sl = slice(i * BL, (i + 1) * BL)
        fTc = np.ascontiguousarray(
            frames[sl].transpose(0, 2, 1)           # [BL, H, Tv]
            .reshape(BL, KT, 128, Tv)
            .transpose(0, 2, 1, 3)                  # [BL, 128, KT, Tv]
        ).astype(NP_FP8)
        tTc = np.ascontiguousarray(
            text[sl].transpose(2, 0, 1)             # [H, BL, Tt]
            .reshape(KT, 128, BL, Tt)
            .transpose(1, 0, 2, 3)                  # [128, KT, BL, Tt]
            .reshape(128, KT, BL * Tt)
        ).astype(NP_FP8)
        hTc = np.ascontiguousarray(
            h[sl].T.reshape(KT, 128, BL).transpose(1, 0, 2)
        )
        hT8c = np.zeros((128, KT, 16), np.float32)
        hT8c[:, :, 0:BL] = hTc
        in_maps.append(dict(
            shared, fT=fTc, tT=tTc,
            hT8=hT8c.astype(NP_FP8),
            hT16=hTc.reshape(128, KT * BL).astype(NP_BF16),
        ))
    return in_maps


def run(inputs, trace=False, **kw):
    nc = _get_nc()
    in_maps = make_in_maps(inputs)
    res = run_bass_kernel_spmd(nc, in_maps, core_ids=list(range(NC)), trace=trace, **kw)
    out = np.concatenate([res.results[i]["out"] for i in range(NC)], axis=0)
    return out, res


def kernel(**inputs) -> np.ndarray:
    out, _ = run(inputs, trace=False)
    return out


# revision 3
# speedup vs baseline: 2.3216x; 1.0892x over previous
"""Trainium2 Bass kernel for nn_AttentionTwoStream — fp8 DoubleRow edition.

Sharding: data-parallel over batch B=64 -> 8 batches/core; all (512,512)
weights replicated. Cross-core coupling: the beta softmax over 2B logits.

Collectives (the dominant cost in this environment): one warmup AllGather
at t=0 absorbs the ~40us ncfw first-collective init plus inter-core launch
skew while compute runs underneath; the two real logit exchanges (visual +
text) are merged into a SINGLE 16-float AllGather at the end.

Compute strategy:
- All large matmuls (frames/text/h-projections/gates) in fp8e4 DoubleRow
  (2 k-subtiles per instruction, ~2x bf16 rate). Weights pre-scaled x64 on
  host (fp8e4 min-normal is 2^-6; raw 0.01-scale weights would be
  subnormal); scales are divided out in downstream activations (x64 for
  score paths, x1024 for gate paths via the 16*I wbs-injection matmul).
- Only h@Whh stays bf16 — it dominates the output; every fp8 path is
  suppressed by the tiny beta gate, keeping rel err ~2.8e-3 (gate: 2e-2).
- tanh/exp on ACT with fused scale + per-partition bias; PSUM evacuations
  and the attention einsums on DVE; av/at broadcasts via PE ones-matmul.
- Frames pairs 0-1 are scheduled before the text stream (their fp8 frames
  shards are the first DMA arrivals), pairs 2-3 after; score projections
  and softmax->einsum chains run one stage behind the main matmuls.

Self-contained: hardcodes B=64, Tv=512, Tt=64, H=512, 8 cores.
"""

import numpy as np
import ml_dtypes

import concourse.bacc as bacc
import concourse.bass as bass
import concourse.mybir as mybir
import concourse.tile as tile
from concourse.bass_utils import run_bass_kernel_spmd

NC = 8
B = 64
BL = B // NC    # 8
H = 512
Tv = 512
Tt = 64
KT = H // 128   # 4
WS = 64.0       # fp8 weight pre-scale
F32 = mybir.dt.float32
BF16 = mybir.dt.bfloat16
FP8 = mybir.dt.float8e4
NP_BF16 = ml_dtypes.bfloat16
NP_FP8 = ml_dtypes.float8_e4m3
DR = mybir.MatmulPerfMode.DoubleRow


def build_nc():
    nc = bacc.Bacc(
        "TRN2", target_bir_lowering=False, debug=False,
        enable_asserts=False, num_devices=NC, num_swdge_queues=4,
    )

    def inp(name, shape, dt):
        return nc.dram_tensor(name, list(shape), dt, kind="ExternalInput").ap()

    # --- external inputs (per-core shards; layouts match SBUF tiles) ---
    fT = inp("fT", (BL, 128, KT, Tv), FP8)     # frames^T [b][k%128][kt][t]
    tT = inp("tT", (128, KT, BL * Tt), FP8)    # text^T
    hT8 = inp("hT8", (128, KT, 16), FP8)   # b-dim padded to 16 (DR stride align)       # h^T fp8
    hT16 = inp("hT16", (128, KT * BL), BF16)   # h^T bf16 (Whh path)
    wavS = inp("wavS", (128, KT, H), FP8)      # 64*Wav
    watS = inp("watS", (128, KT, H), FP8)
    uavS = inp("uavS", (128, KT, H), FP8)
    uatS = inp("uatS", (128, KT, H), FP8)
    wb8 = inp("wb8", (128, KT, H), FP8)        # 64*Wb
    whh = inp("whh", (128, KT * H), BF16)
    wveT8 = inp("wveT8", (128, KT, H), FP8)    # 64*Wve.T
    wqeT8 = inp("wqeT8", (128, KT, H), FP8)
    vbv8 = inp("vbv8", (128, KT, H), FP8)
    vbt8 = inp("vbt8", (128, KT, H), FP8)
    eyeD = inp("eyeD", (BL, BL), BF16)         # 16*I (wbs scale injection)
    vavZ = inp("vavZ", (128, KT, 16), FP8)      # [p,jt,i*2+m]: m==i -> 64*Vav
    vatZ = inp("vatZ", (128, KT, 16), FP8)      # 64*Vat
    biasr = inp("biasr", (1, 8 * H), BF16)     # 64*bav,64*bat,bve,bqe,bbv,bbt,bh
    wbB = inp("wbB", (BL, H), F32)             # wb broadcast to 8 partitions
    eye = inp("eye", (128, 128), BF16)
    blkI = inp("blkI", (BL, BL * Tt), BF16)    # blkI[b, b'*64+t] = (b == b')

    out_ext = nc.dram_tensor("out", [BL, H], F32, kind="ExternalOutput").ap()

    ACT = mybir.ActivationFunctionType
    ALU = mybir.AluOpType
    R64 = 1.0 / WS

    with tile.TileContext(nc) as tc:
        with (
            tc.tile_pool(name="wres", bufs=1) as wres,
            tc.tile_pool(name="work", bufs=2) as work,
            tc.tile_pool(name="small", bufs=1) as small,
            tc.tile_pool(name="psX", bufs=3, space="PSUM") as psX,
            tc.tile_pool(name="psS", bufs=2, space="PSUM") as psS,
            tc.tile_pool(name="psB", bufs=1, space="PSUM") as psB,
            tc.tile_pool(name="psG", bufs=2, space="PSUM") as psG,
            tc.tile_pool(name="dram", bufs=1, space="DRAM") as dram,
        ):
            # ---------- warmup collective at t=0 (absorbs ncfw init + skew) --
            warm_out = dram.tile([2 * B, 1], F32, tag="warmout", addr_space="Shared")
            warm_in = dram.tile([2 * BL, 1], F32, tag="warmin")
            nc.gpsimd.collective_compute(
                "AllGather", ALU.bypass,
                replica_groups=[list(range(NC))],
                ins=[warm_in[:].opt()],
                outs=[warm_out[:].opt()],
            )

            # ---------- PE warmup on a memset tile (no DMA dependency) -------
            warmA = small.tile([128, 128], BF16, tag="warmA")
            nc.vector.memset(warmA[:], 0.001)
            warm_ps = psB.tile([128, 128], F32, tag="psB", name="warmps")
            for _ in range(28):
                nc.tensor.matmul(
                    warm_ps[:], warmA[:], warmA[:],
                    start=True, stop=True, skip_group_check=True,
                )

            # ---------- DMAs (priority order) --------------------------------
            def load(pool, ap_in, shape, dt, tag, engine, name=None):
                t = pool.tile(list(shape), dt, tag=tag, name=name or tag)
                engine.dma_start(t[:], ap_in)
                return t

            fT_sb = [None] * BL
            def load_fT(b, eng):
                t = wres.tile([128, KT, Tv], FP8, tag=f"fT{b}", name=f"fTs{b}")
                eng.dma_start(t[:], fT[b])
                fT_sb[b] = t
            # sync queue: P1 controls then fT0/fT1 (fastest start)
            hT8_sb = load(wres, hT8, (128, KT, 16), FP8, "hT8", nc.sync)
            biasr_sb = load(wres, biasr, (1, 8 * H), BF16, "biasr", nc.sync)
            eye_sb = load(wres, eye, (128, 128), BF16, "eye", nc.sync)
            vavZ_sb = load(wres, vavZ, (128, KT, 16), FP8, "vavZ", nc.sync)
            load_fT(0, nc.sync)
            load_fT(1, nc.sync)
            hT16_sb = load(wres, hT16, (128, KT * BL), BF16, "hT16", nc.sync)
            wbB_sb = load(wres, wbB, (BL, H), F32, "wbB", nc.sync)
            blkI_sb = load(wres, blkI, (BL, BL * Tt), BF16, "blkI", nc.sync)
            vatZ_sb = load(wres, vatZ, (128, KT, 16), FP8, "vatZ", nc.sync)
            eyeD_sb = load(wres, eyeD, (BL, BL), BF16, "eyeD", nc.sync)
            wb8_sb = load(wres, wb8, (128, KT, H), FP8, "wb8", nc.sync)
            vbt8_sb = load(wres, vbt8, (128, KT, H), FP8, "vbt8", nc.sync)
            wqeT8_sb = load(wres, wqeT8, (128, KT, H), FP8, "wqeT8", nc.sync)
            # scalar queue: P1 + frames weights first, late weights after
            uavS_sb = load(wres, uavS, (128, KT, H), FP8, "uavS", nc.scalar)
            wavS_sb = load(wres, wavS, (128, KT, H), FP8, "wavS", nc.scalar)
            uatS_sb = load(wres, uatS, (128, KT, H), FP8, "uatS", nc.scalar)
            watS_sb = load(wres, watS, (128, KT, H), FP8, "watS", nc.scalar)
            whh_sb = load(wres, whh, (128, KT * H), BF16, "whh", nc.scalar)
            vbv8_sb = load(wres, vbv8, (128, KT, H), FP8, "vbv8", nc.scalar)
            wveT8_sb = load(wres, wveT8, (128, KT, H), FP8, "wveT8", nc.scalar)
            # gpsimd: remaining frames + text
            load_fT(2, nc.gpsimd)
            load_fT(3, nc.gpsimd)
            tT_sb = load(wres, tT, (128, KT, BL * Tt), FP8, "tT", nc.gpsimd)
            for b in range(4, BL):
                load_fT(b, nc.gpsimd)

            ones_sb = small.tile([1, 128], BF16, tag="ones")
            nc.vector.memset(ones_sb[:], 1.0)

            def brow(i):
                return biasr_sb[0:1, i * H:(i + 1) * H]
            bavS_r, batS_r, bve_r, bqe_r, bbv_r, bbt_r, bh_r = (brow(i) for i in range(7))

            # fp8 DoubleRow gate: out[8,512] = 64*(h@W + bias);  2 DR + 1 bias mm
            def gate8(w_sb, bias_row, name):
                ps = psG.tile([BL, H], F32, tag="psG", name=name)
                for q in range(2):
                    nc.tensor.matmul(
                        ps[:], hT8_sb[:, 2 * q:2 * q + 2, 0:BL],
                        w_sb[:, 2 * q:2 * q + 2, :],
                        start=(q == 0), stop=False, perf_mode=DR,
                        skip_group_check=True,
                    )
                nc.tensor.matmul(
                    ps[:], ones_sb[0:1, 0:BL], bias_row,
                    start=False, stop=True, skip_group_check=True,
                )
                return ps

            # bf16 gate: accum into ps; lhsT [128, 8] kt-sliced
            def gate16(lhsT_sb, w_sb, bias_row, ps=None, name="g16",
                       start=True, stop=True):
                if ps is None:
                    ps = psG.tile([BL, H], F32, tag="psG", name=name)
                for kt in range(KT):
                    nc.tensor.matmul(
                        ps[:], lhsT_sb[:, kt * BL:(kt + 1) * BL],
                        w_sb[:, kt * H:(kt + 1) * H],
                        start=(start and kt == 0), stop=False,
                        skip_group_check=True,
                    )
                nc.tensor.matmul(
                    ps[:], ones_sb[0:1, 0:BL], bias_row,
                    start=False, stop=stop, skip_group_check=True,
                )
                return ps

            # ---------- P1: h projections (fp8, x64-scaled) ------------------
            uhvb_ps = gate8(uavS_sb, bavS_r, "uhvb")     # 64*(Uhv + bav)
            uhtb_ps = gate8(uatS_sb, batS_r, "uhtb")

            uhvb_s = small.tile([BL, H], BF16, tag="uhvb_s")   # scaled
            nc.vector.tensor_copy(uhvb_s[:], uhvb_ps[:])
            uhtb_s = small.tile([BL, H], BF16, tag="uhtb_s")   # scaled
            nc.vector.tensor_copy(uhtb_s[:], uhtb_ps[:])

            # uhvbT: [j, (jt,b)] UNSCALED f32 for ACT bias
            uhvbT_sb = small.tile([128, KT * BL], F32, tag="uhvbT")
            for jt in range(KT):
                tp = psB.tile([128, BL], BF16, tag="psB", name=f"tpv{jt}")
                nc.tensor.transpose(
                    tp[:], uhvb_s[0:BL, jt * 128:(jt + 1) * 128],
                    eye_sb[0:BL, 0:BL],
                )
                nc.vector.tensor_scalar_mul(
                    uhvbT_sb[:, jt * BL:(jt + 1) * BL], tp[:], R64,
                )

            # ---------- frames stream state ----------
            hvT_sb = small.tile([128, KT * BL], F32, tag="hvT")
            NP_ = BL // 2
            yv_tiles = {}
            scv_tiles = {}

            def frames_stage(g, jt):
                """Emit main DR matmuls for (pair g, out-tile jt)."""
                bs = (2 * g, 2 * g + 1)
                xps = [psX.tile([128, Tv], F32, tag="psX", name=f"xp{g}_{jt}_{i}")
                       for i in range(2)]
                for q in range(2):
                    for i, b in enumerate(bs):
                        nc.tensor.matmul(
                            xps[i][:],
                            wavS_sb[:, 2 * q:2 * q + 2, jt * 128:(jt + 1) * 128],
                            fT_sb[b][:, 2 * q:2 * q + 2, :],
                            start=(q == 0), stop=(q == 1), perf_mode=DR,
                        )
                return xps

            def frames_tanh(g, jt, xps):
                if (g, 0) not in yv_tiles:
                    for qq in range(2):
                        yv_tiles[(g, qq)] = work.tile(
                            [128, 2, 2, Tv], FP8, tag="yv4", name=f"yv4_{g}_{qq}",
                            bufs=4,
                        )
                q, jj = jt // 2, jt % 2
                for i, b in enumerate((2 * g, 2 * g + 1)):
                    nc.scalar.activation(
                        yv_tiles[(g, q)][:, jj, i, :], xps[i][:], ACT.Tanh,
                        bias=uhvbT_sb[:, jt * BL + b: jt * BL + b + 1],
                        scale=R64,
                    )

            def scv_mm(g, q):
                """Score DR matmuls for jt-pair q of pair g."""
                if g not in scv_tiles:
                    scv_tiles[g] = psS.tile([2, Tv], F32, tag="psS", name=f"scv{g}")
                scv_g = scv_tiles[g]
                yv4 = yv_tiles[(g, q)]
                for i in range(2):
                    nc.tensor.matmul(
                        scv_g[:], vavZ_sb[:, 2 * q:2 * q + 2, 2 * i:2 * i + 2],
                        yv4[:, :, i, :],
                        start=(q == 0 and i == 0), stop=(q == 1 and i == 1),
                        perf_mode=DR, skip_group_check=True,
                    )

            def pair_chain(g):
                """softmax -> DRAM-broadcast -> einsum for pair g."""
                bs = (2 * g, 2 * g + 1)
                scv_g = scv_tiles[g]
                expv = small.tile([2, Tv], F32, tag="expv", name=f"expv{g}", bufs=2)
                sumv = small.tile([2, 1], F32, tag="sumv", name=f"sumv{g}", bufs=2)
                nc.scalar.activation(
                    expv[:], scv_g[:], ACT.Exp, scale=R64, accum_out=sumv[:],
                )
                rv = small.tile([2, 1], F32, tag="rv", name=f"rv{g}", bufs=2)
                nc.vector.reciprocal(rv[:], sumv[:])
                avp = small.tile([2, Tv], BF16, tag="av", name=f"av{g}", bufs=2)
                nc.vector.tensor_scalar_mul(avp[:], expv[:], rv[:])
                avR = small.tile([1, 2 * Tv], BF16, tag="avR", name=f"avR{g}", bufs=2)
                nc.sync.dma_start(avR[0:1, :], avp[:, :])
                scr = work.tile([128, Tv], BF16, tag="scr", name=f"scr{g}", bufs=2)
                for i, b in enumerate(bs):
                    avB_ps = psB.tile([128, Tv], F32, tag="psB", name=f"avB{b}")
                    bsrc = avp[0:1, :] if i == 0 else avR[0:1, Tv:2 * Tv]
                    nc.tensor.matmul(
                        avB_ps[:], ones_sb[0:1, 0:128], bsrc,
                        start=True, stop=True, skip_group_check=True,
                    )
                    avB = work.tile([128, Tv], BF16, tag="avB", name=f"avB{g}_{i}",
                                    bufs=4)
                    nc.vector.tensor_copy(avB[:], avB_ps[:])
                    for kt in range(KT):
                        nc.vector.scalar_tensor_tensor(
                            out=scr[:],
                            in0=fT_sb[b][:, kt, :],
                            scalar=1.0,
                            in1=avB[:],
                            op0=ALU.mult, op1=ALU.mult,
                            accum_out=hvT_sb[:, kt * BL + b: kt * BL + b + 1],
                        )

            # ---------- frames pairs 0-1 (earliest DMA arrivals) ----------
            for g in range(2):
                for jt in range(KT):
                    xps = frames_stage(g, jt)
                    if g == 1 and jt == 0:
                        scv_mm(0, 0)
                    elif g == 1 and jt == 2:
                        scv_mm(0, 1)
                        pair_chain(0)
                    frames_tanh(g, jt, xps)

            # ---------- text stream (fp8 DR): Xt' = 64*(Wat.T@tT + Uht + bat)
            yt4 = small.tile([128, KT, BL * Tt], FP8, tag="yt4")
            for jt in range(KT):
                xt_ps = psX.tile([128, BL * Tt], F32, tag="psX", name=f"xt{jt}")
                for q in range(2):
                    nc.tensor.matmul(
                        xt_ps[:],
                        watS_sb[:, 2 * q:2 * q + 2, jt * 128:(jt + 1) * 128],
                        tT_sb[:, 2 * q:2 * q + 2, :],
                        start=(q == 0), stop=False, perf_mode=DR,
                        skip_group_check=True,
                    )
                nc.tensor.matmul(      # + 64*(Uht+bat) via block identity
                    xt_ps[:], uhtb_s[0:BL, jt * 128:(jt + 1) * 128], blkI_sb[:],
                    start=False, stop=True, skip_group_check=True,
                )
                nc.scalar.activation(yt4[:, jt, :], xt_ps[:], ACT.Tanh, scale=R64)

            scv_mm(1, 0)

            # hWhh + bh (bf16)
            hwhh_ps = gate16(hT16_sb, whh_sb, bh_r, name="hwhh")
            hwhh_sb = small.tile([BL, H], F32, tag="hwhh_sb")
            nc.vector.tensor_copy(hwhh_sb[:], hwhh_ps[:])

            # sct: scores_t = 64 * yt @ Vat   (fp8 DR)
            sct_ps = psS.tile([1, BL * Tt], F32, tag="psS", name="sct")
            for q in range(2):
                nc.tensor.matmul(
                    sct_ps[:], vatZ_sb[:, 2 * q:2 * q + 2, 0:1],
                    yt4[:, 2 * q:2 * q + 2, :],
                    start=(q == 0), stop=(q == 1), perf_mode=DR,
                    skip_group_check=True,
                )

            # ---------- text softmax ----------
            sct_sb = small.tile([1, BL * Tt], F32, tag="sct_sb")
            nc.vector.tensor_copy(sct_sb[:], sct_ps[:])
            st8 = small.tile([BL, Tt], F32, tag="st8")
            nc.sync.dma_start(st8[:, :], sct_sb[0:1, :])
            expt_sb = small.tile([BL, Tt], F32, tag="expt")
            sumt = small.tile([BL, 1], F32, tag="sumt")
            nc.scalar.activation(
                expt_sb[:], st8[:], ACT.Exp, scale=R64, accum_out=sumt[:],
            )
            rt = small.tile([BL, 1], F32, tag="rt")
            nc.vector.reciprocal(rt[:], sumt[:])
            at_sb = small.tile([BL, Tt], BF16, tag="at")
            nc.vector.tensor_scalar_mul(at_sb[:], expt_sb[:], rt[:])

            # atB: broadcast at across partitions via PE ones-matmul
            atRows = small.tile([1, BL * Tt], BF16, tag="atRows")
            nc.sync.dma_start(atRows[0:1, :], at_sb[:, :])
            atB_ps = psB.tile([128, BL * Tt], F32, tag="psB", name="atB")
            for b in range(BL):
                bsrc = at_sb[0:1, :] if b == 0 else atRows[0:1, b * Tt:(b + 1) * Tt]
                nc.tensor.matmul(
                    atB_ps[:, b * Tt:(b + 1) * Tt], ones_sb[0:1, 0:128], bsrc,
                    start=True, stop=True, skip_group_check=True,
                )
            atB_sb = small.tile([128, BL * Tt], BF16, tag="atB_sb")
            nc.vector.tensor_copy(atB_sb[:], atB_ps[:])

            # text einsum on DVE: htT[:, kt*8+b] = sum_t tT*at
            htT_sb = small.tile([128, KT * BL], F32, tag="htT")
            scrt = small.tile([128, Tt], BF16, tag="scrt")
            for kt in range(KT):
                for b in range(BL):
                    nc.vector.scalar_tensor_tensor(
                        out=scrt[:],
                        in0=tT_sb[:, kt, b * Tt:(b + 1) * Tt],
                        scalar=1.0,
                        in1=atB_sb[:, b * Tt:(b + 1) * Tt],
                        op0=ALU.mult, op1=ALU.mult,
                        accum_out=htT_sb[:, kt * BL + b: kt * BL + b + 1],
                    )

            # ---------- text gates + lgt into cc_in (all x1024-scaled) -------
            # wbs = 64*(h@Wb)  (shared by mt1 and mv1)
            wbs_ps = psG.tile([BL, H], F32, tag="psG", name="wbs")
            for q in range(2):
                nc.tensor.matmul(
                    wbs_ps[:], hT8_sb[:, 2 * q:2 * q + 2, 0:BL],
                    wb8_sb[:, 2 * q:2 * q + 2, :],
                    start=(q == 0), stop=(q == 1), perf_mode=DR,
                    skip_group_check=True,
                )
            wbs_sb = small.tile([BL, H], BF16, tag="wbs_sb")
            nc.vector.tensor_copy(wbs_sb[:], wbs_ps[:])

            def gate1024(lhsT8, w8, bias_row, name):
                """x1024 group: 16*eyeD@wbs(64x) + bias(1024x) + lhsT8@w8."""
                ps = psG.tile([BL, H], F32, tag="psG", name=name)
                nc.tensor.matmul(
                    ps[:], eyeD_sb[:], wbs_sb[:],
                    start=True, stop=False, skip_group_check=True,
                )
                nc.tensor.matmul(
                    ps[:], ones_sb[0:1, 0:BL], bias_row,
                    start=False, stop=False, skip_group_check=True,
                )
                for q in range(2):
                    nc.tensor.matmul(
                        ps[:], lhsT8[:, 2 * q:2 * q + 2, 0:BL],
                        w8[:, 2 * q:2 * q + 2, :],
                        start=False, stop=(q == 1), perf_mode=DR,
                        skip_group_check=True,
                    )
                return ps

            def gate1024_nb(lhsT8, w8, bias_row, name):
                """x1024 group without wbs: bias(1024x) + lhsT8@w8."""
                ps = psG.tile([BL, H], F32, tag="psG", name=name)
                nc.tensor.matmul(
                    ps[:], ones_sb[0:1, 0:BL], bias_row,
                    start=True, stop=False, skip_group_check=True,
                )
                for q in range(2):
                    nc.tensor.matmul(
                        ps[:], lhsT8[:, 2 * q:2 * q + 2, 0:BL],
                        w8[:, 2 * q:2 * q + 2, :],
                        start=False, stop=(q == 1), perf_mode=DR,
                        skip_group_check=True,
                    )
                return ps

            # htT8: 16*ht_sum in fp8, padded [128, KT, 16]
            htT8 = small.tile([128, KT, 16], FP8, tag="htT8")
            for kt in range(KT):
                nc.vector.tensor_scalar_mul(
                    htT8[:, kt, 0:BL], htT_sb[:, kt * BL:(kt + 1) * BL], 16.0,
                )
            mt1_ps = gate1024(htT8, vbt8_sb, bbt_r, "mt1")
            ht2_ps = gate1024_nb(htT8, wqeT8_sb, bqe_r, "ht2")

            scv_mm(1, 1)
            pair_chain(1)

            lgt = small.tile([BL, 1], F32, tag="lgt")
            g_sb = small.tile([1, 2 * B], F32, tag="g")
            cc_in = dram.tile([2 * BL, 1], F32, tag="ccin")
            cc_out = dram.tile([2 * B, 1], F32, tag="ccout", addr_space="Shared")

            mtv_t = small.tile([BL, H], F32, tag="mtv_t")
            nc.scalar.activation(mtv_t[:], mt1_ps[:], ACT.Tanh, scale=1.0 / 1024.0)
            scr8b = small.tile([BL, H], F32, tag="scr8b")
            nc.vector.scalar_tensor_tensor(
                out=scr8b[:], in0=mtv_t[:], scalar=1.0, in1=wbB_sb[:],
                op0=ALU.mult, op1=ALU.mult, accum_out=lgt[:],
            )
            nc.sync.dma_start(cc_in[BL:2 * BL, :], lgt[:])
            # evacuate ht2 early (frees its PSUM bank before mv1/hv2)
            ht2_sb = small.tile([BL, H], F32, tag="ht2_sb")
            nc.vector.tensor_scalar_mul(ht2_sb[:], ht2_ps[:], 1.0 / 1024.0)

            # ---------- frames pairs 2-3 ----------
            for g in range(2, NP_):
                for jt in range(KT):
                    xps = frames_stage(g, jt)
                    if g == 3 and jt == 0:
                        scv_mm(2, 0)
                    elif g == 3 and jt == 2:
                        scv_mm(2, 1)
                        pair_chain(2)
                    frames_tanh(g, jt, xps)

            scv_mm(NP_ - 1, 0)
            scv_mm(NP_ - 1, 1)
            pair_chain(NP_ - 1)

            # ---------- visual gates (fp8, x1024) ----------
            hvT8 = small.tile([128, KT, 16], FP8, tag="hvT8")
            for kt in range(KT):
                nc.vector.tensor_scalar_mul(
                    hvT8[:, kt, 0:BL], hvT_sb[:, kt * BL:(kt + 1) * BL], 16.0,
                )
            mv1_ps = gate1024(hvT8, vbv8_sb, bbv_r, "mv1")
            hv2_ps = gate1024_nb(hvT8, wveT8_sb, bve_r, "hv2")

            mtv_v = small.tile([BL, H], F32, tag="mtv_v")
            nc.scalar.activation(mtv_v[:], mv1_ps[:], ACT.Tanh, scale=1.0 / 1024.0)
            lgv = small.tile([BL, 1], F32, tag="lgv")
            scr8 = small.tile([BL, H], F32, tag="scr8")
            nc.vector.scalar_tensor_tensor(
                out=scr8[:], in0=mtv_v[:], scalar=1.0, in1=wbB_sb[:],
                op0=ALU.mult, op1=ALU.mult, accum_out=lgv[:],
            )

            nc.sync.dma_start(cc_in[0:BL, :], lgv[:])
            nc.gpsimd.collective_compute(
                "AllGather", ALU.bypass,
                replica_groups=[list(range(NC))],
                ins=[cc_in[:].opt()],
                outs=[cc_out[:].opt()],
            )

            # ---------- global beta softmax + output ----------
            # g_sb layout: [core0: lgv(8), lgt(8); core1: ...] — order is
            # irrelevant for the softmax sum; beta0/1 = exp(g[0:2])/sum.
            nc.sync.dma_start(g_sb[0:1, :], cc_out[:, :])
            ge_sb = small.tile([1, 2 * B], F32, tag="ge")
            sumg = small.tile([1, 1], F32, tag="sumg")
            nc.scalar.activation(ge_sb[:], g_sb[:], ACT.Exp, accum_out=sumg[:])
            rg = small.tile([1, 1], F32, tag="rg")
            nc.vector.reciprocal(rg[:], sumg[:])
            betas = small.tile([1, 2], BF16, tag="betas")
            nc.vector.tensor_scalar_mul(betas[:], ge_sb[0:1, 0:2], rg[:])
            beta8_ps = psB.tile([BL, 2], F32, tag="psB", name="beta8")
            nc.tensor.matmul(
                beta8_ps[:], ones_sb[0:1, 0:BL], betas[0:1, 0:2],
                start=True, stop=True, skip_group_check=True,
            )
            hv2_sb = small.tile([BL, H], F32, tag="hv2_sb")
            nc.vector.tensor_scalar_mul(hv2_sb[:], hv2_ps[:], 1.0 / 1024.0)
            t1 = small.tile([BL, H], F32, tag="t1")
            nc.vector.scalar_tensor_tensor(
                out=t1[:], in0=hv2_sb[:], scalar=beta8_ps[:, 0:1], in1=hwhh_sb[:],
                op0=ALU.mult, op1=ALU.add,
            )
            s1 = small.tile([BL, H], F32, tag="s1")
            nc.vector.scalar_tensor_tensor(
                out=s1[:], in0=ht2_sb[:], scalar=beta8_ps[:, 1:2], in1=t1[:],
                op0=ALU.mult, op1=ALU.add,
            )
            out_sb = small.tile([BL, H], F32, tag="out_sb")
            nc.scalar.activation(out_sb[:], s1[:], ACT.Tanh)
            nc.sync.dma_start(out_ext, out_sb[:])

    nc.compile()
    return nc


_cached_nc = None


def _get_nc():
    global _cached_nc
    if _cached_nc is None:
        _cached_nc = build_nc()
    return _cached_nc


def _pack_w8(w, scale=WS):
    """[512,512] -> [128, KT, H] fp8, [p, kt, j] = scale*w[kt*128+p, j]."""
    a = np.asarray(w, np.float32).reshape(KT, 128, H).transpose(1, 0, 2) * scale
    return np.ascontiguousarray(a).astype(NP_FP8)


def _pack_w16(w):
    a = np.asarray(w, np.float32).reshape(KT, 128, H).transpose(1, 0, 2)
    return np.ascontiguousarray(a.reshape(128, KT * H)).astype(NP_BF16)


def make_in_maps(inputs):
    h = np.asarray(inputs["h"], np.float32)
    frames = np.asarray(inputs["hidden_frames"], np.float32)
    text = np.asarray(inputs["hidden_text"], np.float32)
    Vav = np.asarray(inputs["Vav"], np.float32)
    Vat = np.asarray(inputs["Vat"], np.float32)
    wb = np.asarray(inputs["wb"], np.float32)

    vavZ = np.zeros((128, KT, 16), np.float32)
    for jt in range(KT):
        for i in range(2):
            vavZ[:, jt, 2 * i + i] = WS * Vav[jt * 128:(jt + 1) * 128]
    vavZ = vavZ.astype(NP_FP8)
    vatZ = np.zeros((128, KT, 16), np.float32)
    vatZ[:, :, 0] = (WS * Vat).reshape(KT, 128).T
    vatZ = vatZ.astype(NP_FP8)

    biasr = np.zeros((1, 8 * H), np.float32)
    for i, (k, sc) in enumerate([
        ("bav", WS), ("bat", WS), ("bve", 1024.0), ("bqe", 1024.0),
        ("bbv", 1024.0), ("bbt", 1024.0), ("bh", 1.0),
    ]):
        biasr[0, i * H:(i + 1) * H] = sc * np.asarray(inputs[k], np.float32)
    biasr = biasr.astype(NP_BF16)
    wbB = np.ascontiguousarray(np.broadcast_to(wb, (BL, H))).astype(np.float32)
    eye = np.eye(128, dtype=np.float32).astype(NP_BF16)
    blkI = np.zeros((BL, BL, Tt), np.float32)
    for b in range(BL):
        blkI[b, b, :] = 1.0
    blkI = blkI.reshape(BL, BL * Tt).astype(NP_BF16)

    eyeD = (16.0 * np.eye(BL, dtype=np.float32)).astype(NP_BF16)
    shared = dict(
        wavS=_pack_w8(inputs["Wav"]), watS=_pack_w8(inputs["Wat"]),
        uavS=_pack_w8(inputs["Uav"]), uatS=_pack_w8(inputs["Uat"]),
        wb8=_pack_w8(inputs["Wb"]), whh=_pack_w16(inputs["Whh"]),
        wveT8=_pack_w8(np.asarray(inputs["Wve"], np.float32).T),
        wqeT8=_pack_w8(np.asarray(inputs["Wqe"], np.float32).T),
        vbv8=_pack_w8(inputs["Vbv"]), vbt8=_pack_w8(inputs["Vbt"]),
        vavZ=vavZ, vatZ=vatZ, biasr=biasr, wbB=wbB, eye=eye, blkI=blkI,
        eyeD=eyeD,
    )

    in_maps = []
    for i in range(NC):
        sl = slice(i * BL, (i + 1) * BL)
        fTc = np.ascontiguousarray(
            frames[sl].transpose(0, 2, 1)           # [BL, H, Tv]
            .reshape(BL, KT, 128, Tv)
            .transpose(0, 2, 1, 3)                  # [BL, 128, KT, Tv]
        ).astype(NP_FP8)
        tTc = np.ascontiguousarray(
            text[sl].transpose(2, 0, 1)             # [H, BL, Tt]
            .reshape(KT, 128, BL, Tt)
            .transpose(1, 0, 2, 3)                  # [128, KT, BL, Tt]
            .reshape(128, KT, BL * Tt)
        ).astype(NP_FP8)
        hTc = np.ascontiguousarray(
            h[sl].T.reshape(KT, 128, BL).transpose(1, 0, 2)
        )
        hT8c = np.zeros((128, KT, 16), np.float32)
        hT8c[:, :, 0:BL] = hTc
        in_maps.append(dict(
            shared, fT=fTc, tT=tTc,
            hT8=hT8c.astype(NP_FP8),
            hT16=hTc.reshape(128, KT * BL).astype(NP_BF16),
        ))
    return in_maps


def run(inputs, trace=False, **kw):
    nc = _get_nc()
    in_maps = make_in_maps(inputs)
    res = run_bass_kernel_spmd(nc, in_maps, core_ids=list(range(NC)), trace=trace, **kw)
    out = np.concatenate([res.results[i]["out"] for i in range(NC)], axis=0)
    return out, res


def kernel(**inputs) -> np.ndarray:
    out, _ = run(inputs, trace=False)
    return out


# revision 4
# speedup vs baseline: 2.3898x; 1.0294x over previous
"""Trainium2 Bass kernel for nn_AttentionTwoStream — fp8 DoubleRow edition.

Sharding: data-parallel over batch B=64 -> 8 batches/core; all (512,512)
weights replicated. Cross-core coupling: the beta softmax over 2B logits.

Collectives (the dominant cost in this environment): one warmup AllGather
at t=0 absorbs the ~40us ncfw first-collective init plus inter-core launch
skew while compute runs underneath; the two real logit exchanges (visual +
text) are merged into a SINGLE 16-float AllGather at the end.

Compute strategy:
- All large matmuls (frames/text/h-projections/gates) in fp8e4 DoubleRow
  (2 k-subtiles per instruction, ~2x bf16 rate). Weights pre-scaled x64 on
  host (fp8e4 min-normal is 2^-6; raw 0.01-scale weights would be
  subnormal); scales are divided out in downstream activations (x64 for
  score paths, x1024 for gate paths via the 16*I wbs-injection matmul).
- Only h@Whh stays bf16 — it dominates the output; every fp8 path is
  suppressed by the tiny beta gate, keeping rel err ~2.8e-3 (gate: 2e-2).
- tanh/exp on ACT with fused scale + per-partition bias; attention einsums
  on DVE reading the softmax-weight broadcast straight from PSUM; pure
  PSUM evacuations on the Scalar engine.
- Frames pairs 0-2 run before the text stream (their fp8 shards are the
  first DMA arrivals), pair 3 after, so only one softmax->einsum chain
  remains in the pre-AllGather tail; score projections and chains run one
  stage behind the main matmuls. psB has 2 PSUM banks so consecutive
  chains' broadcast/einsum overlap; the gate pool (psG) gets 1 bank, which
  only serializes ht2/hv2 — tensors not needed until after the collective.

Self-contained: hardcodes B=64, Tv=512, Tt=64, H=512, 8 cores.
"""

import numpy as np
import ml_dtypes

import concourse.bacc as bacc
import concourse.bass as bass
import concourse.mybir as mybir
import concourse.tile as tile
from concourse.bass_utils import run_bass_kernel_spmd

NC = 8
B = 64
BL = B // NC    # 8
H = 512
Tv = 512
Tt = 64
KT = H // 128   # 4
WS = 64.0       # fp8 weight pre-scale
F32 = mybir.dt.float32
BF16 = mybir.dt.bfloat16
FP8 = mybir.dt.float8e4
NP_BF16 = ml_dtypes.bfloat16
NP_FP8 = ml_dtypes.float8_e4m3
DR = mybir.MatmulPerfMode.DoubleRow


def build_nc():
    nc = bacc.Bacc(
        "TRN2", target_bir_lowering=False, debug=False,
        enable_asserts=False, num_devices=NC, num_swdge_queues=4,
    )

    def inp(name, shape, dt):
        return nc.dram_tensor(name, list(shape), dt, kind="ExternalInput").ap()

    # --- external inputs (per-core shards; layouts match SBUF tiles) ---
    fT = inp("fT", (BL, 128, KT, Tv), FP8)     # frames^T [b][k%128][kt][t]
    tT = inp("tT", (128, KT, BL * Tt), FP8)    # text^T
    hT8 = inp("hT8", (128, KT, 16), FP8)   # b-dim padded to 16 (DR stride align)       # h^T fp8
    hT16 = inp("hT16", (128, KT * BL), BF16)   # h^T bf16 (Whh path)
    wavS = inp("wavS", (128, KT, H), FP8)      # 64*Wav
    watS = inp("watS", (128, KT, H), FP8)
    uavS = inp("uavS", (128, KT, H), FP8)
    uatS = inp("uatS", (128, KT, H), FP8)
    wb8 = inp("wb8", (128, KT, H), FP8)        # 64*Wb
    whh = inp("whh", (128, KT * H), BF16)
    wveT8 = inp("wveT8", (128, KT, H), FP8)    # 64*Wve.T
    wqeT8 = inp("wqeT8", (128, KT, H), FP8)
    vbv8 = inp("vbv8", (128, KT, H), FP8)
    vbt8 = inp("vbt8", (128, KT, H), FP8)
    eyeD = inp("eyeD", (BL, BL), BF16)         # 16*I (wbs scale injection)
    vavZ = inp("vavZ", (128, KT, 16), FP8)      # [p,jt,i*2+m]: m==i -> 64*Vav
    vatZ = inp("vatZ", (128, KT, 16), FP8)      # 64*Vat
    biasr = inp("biasr", (1, 8 * H), BF16)     # 64*bav,64*bat,bve,bqe,bbv,bbt,bh
    wbB = inp("wbB", (BL, H), F32)             # wb broadcast to 8 partitions
    eye = inp("eye", (128, 128), BF16)
    blkI = inp("blkI", (BL, BL * Tt), BF16)    # blkI[b, b'*64+t] = (b == b')

    out_ext = nc.dram_tensor("out", [BL, H], F32, kind="ExternalOutput").ap()

    ACT = mybir.ActivationFunctionType
    ALU = mybir.AluOpType
    R64 = 1.0 / WS

    with tile.TileContext(nc) as tc:
        with (
            tc.tile_pool(name="wres", bufs=1) as wres,
            tc.tile_pool(name="work", bufs=2) as work,
            tc.tile_pool(name="small", bufs=1) as small,
            tc.tile_pool(name="psX", bufs=3, space="PSUM") as psX,
            tc.tile_pool(name="psS", bufs=2, space="PSUM") as psS,
            tc.tile_pool(name="psB", bufs=2, space="PSUM") as psB,
            tc.tile_pool(name="psG", bufs=1, space="PSUM") as psG,
            tc.tile_pool(name="dram", bufs=1, space="DRAM") as dram,
        ):
            # ---------- warmup collective at t=0 (absorbs ncfw init + skew) --
            warm_out = dram.tile([2 * B, 1], F32, tag="warmout", addr_space="Shared")
            warm_in = dram.tile([2 * BL, 1], F32, tag="warmin")
            nc.gpsimd.collective_compute(
                "AllGather", ALU.bypass,
                replica_groups=[list(range(NC))],
                ins=[warm_in[:].opt()],
                outs=[warm_out[:].opt()],
            )

            # ---------- PE warmup on a memset tile (no DMA dependency) -------
            warmA = small.tile([128, 128], BF16, tag="warmA")
            nc.vector.memset(warmA[:], 0.001)
            warm_ps = psB.tile([128, 128], F32, tag="psB", name="warmps")
            for _ in range(28):
                nc.tensor.matmul(
                    warm_ps[:], warmA[:], warmA[:],
                    start=True, stop=True, skip_group_check=True,
                )

            # ---------- DMAs (priority order) --------------------------------
            def load(pool, ap_in, shape, dt, tag, engine, name=None):
                t = pool.tile(list(shape), dt, tag=tag, name=name or tag)
                engine.dma_start(t[:], ap_in)
                return t

            fT_sb = [None] * BL
            def load_fT(b, eng):
                t = wres.tile([128, KT, Tv], FP8, tag=f"fT{b}", name=f"fTs{b}")
                eng.dma_start(t[:], fT[b])
                fT_sb[b] = t
            # sync queue: P1 controls then fT0/fT1 (fastest start)
            hT8_sb = load(wres, hT8, (128, KT, 16), FP8, "hT8", nc.sync)
            biasr_sb = load(wres, biasr, (1, 8 * H), BF16, "biasr", nc.sync)
            eye_sb = load(wres, eye, (128, 128), BF16, "eye", nc.sync)
            vavZ_sb = load(wres, vavZ, (128, KT, 16), FP8, "vavZ", nc.sync)
            load_fT(0, nc.sync)
            load_fT(1, nc.sync)
            hT16_sb = load(wres, hT16, (128, KT * BL), BF16, "hT16", nc.sync)
            wbB_sb = load(wres, wbB, (BL, H), F32, "wbB", nc.sync)
            blkI_sb = load(wres, blkI, (BL, BL * Tt), BF16, "blkI", nc.sync)
            vatZ_sb = load(wres, vatZ, (128, KT, 16), FP8, "vatZ", nc.sync)
            eyeD_sb = load(wres, eyeD, (BL, BL), BF16, "eyeD", nc.sync)
            wb8_sb = load(wres, wb8, (128, KT, H), FP8, "wb8", nc.sync)
            vbt8_sb = load(wres, vbt8, (128, KT, H), FP8, "vbt8", nc.sync)
            wqeT8_sb = load(wres, wqeT8, (128, KT, H), FP8, "wqeT8", nc.sync)
            # scalar queue: P1 + frames weights first, late weights after
            uavS_sb = load(wres, uavS, (128, KT, H), FP8, "uavS", nc.scalar)
            wavS_sb = load(wres, wavS, (128, KT, H), FP8, "wavS", nc.scalar)
            uatS_sb = load(wres, uatS, (128, KT, H), FP8, "uatS", nc.scalar)
            watS_sb = load(wres, watS, (128, KT, H), FP8, "watS", nc.scalar)
            whh_sb = load(wres, whh, (128, KT * H), BF16, "whh", nc.scalar)
            vbv8_sb = load(wres, vbv8, (128, KT, H), FP8, "vbv8", nc.scalar)
            wveT8_sb = load(wres, wveT8, (128, KT, H), FP8, "wveT8", nc.scalar)
            # gpsimd: remaining frames + text
            load_fT(2, nc.gpsimd)
            load_fT(3, nc.gpsimd)
            tT_sb = load(wres, tT, (128, KT, BL * Tt), FP8, "tT", nc.gpsimd)
            for b in range(4, BL):
                load_fT(b, nc.gpsimd)

            ones_sb = small.tile([1, 128], BF16, tag="ones")
            nc.vector.memset(ones_sb[:], 1.0)

            def brow(i):
                return biasr_sb[0:1, i * H:(i + 1) * H]
            bavS_r, batS_r, bve_r, bqe_r, bbv_r, bbt_r, bh_r = (brow(i) for i in range(7))

            # fp8 DoubleRow gate: out[8,512] = 64*(h@W + bias);  2 DR + 1 bias mm
            def gate8(w_sb, bias_row, name):
                ps = psG.tile([BL, H], F32, tag="psG", name=name)
                for q in range(2):
                    nc.tensor.matmul(
                        ps[:], hT8_sb[:, 2 * q:2 * q + 2, 0:BL],
                        w_sb[:, 2 * q:2 * q + 2, :],
                        start=(q == 0), stop=False, perf_mode=DR,
                        skip_group_check=True,
                    )
                nc.tensor.matmul(
                    ps[:], ones_sb[0:1, 0:BL], bias_row,
                    start=False, stop=True, skip_group_check=True,
                )
                return ps

            # bf16 gate: accum into ps; lhsT [128, 8] kt-sliced
            def gate16(lhsT_sb, w_sb, bias_row, ps=None, name="g16",
                       start=True, stop=True):
                if ps is None:
                    ps = psG.tile([BL, H], F32, tag="psG", name=name)
                for kt in range(KT):
                    nc.tensor.matmul(
                        ps[:], lhsT_sb[:, kt * BL:(kt + 1) * BL],
                        w_sb[:, kt * H:(kt + 1) * H],
                        start=(start and kt == 0), stop=False,
                        skip_group_check=True,
                    )
                nc.tensor.matmul(
                    ps[:], ones_sb[0:1, 0:BL], bias_row,
                    start=False, stop=stop, skip_group_check=True,
                )
                return ps

            # ---------- P1: h projections (fp8, x64-scaled) ------------------
            uhvb_ps = gate8(uavS_sb, bavS_r, "uhvb")     # 64*(Uhv + bav)
            uhtb_ps = gate8(uatS_sb, batS_r, "uhtb")

            uhvb_s = small.tile([BL, H], BF16, tag="uhvb_s")   # scaled
            nc.vector.tensor_copy(uhvb_s[:], uhvb_ps[:])
            uhtb_s = small.tile([BL, H], BF16, tag="uhtb_s")   # scaled
            nc.vector.tensor_copy(uhtb_s[:], uhtb_ps[:])

            # uhvbT: [j, (jt,b)] UNSCALED f32 for ACT bias
            uhvbT_sb = small.tile([128, KT * BL], F32, tag="uhvbT")
            for jt in range(KT):
                tp = psB.tile([128, BL], BF16, tag="psB", name=f"tpv{jt}")
                nc.tensor.transpose(
                    tp[:], uhvb_s[0:BL, jt * 128:(jt + 1) * 128],
                    eye_sb[0:BL, 0:BL],
                )
                nc.vector.tensor_scalar_mul(
                    uhvbT_sb[:, jt * BL:(jt + 1) * BL], tp[:], R64,
                )

            # ---------- frames stream state ----------
            hvT_sb = small.tile([128, KT * BL], F32, tag="hvT")
            NP_ = BL // 2
            yv_tiles = {}
            scv_tiles = {}

            def frames_stage(g, jt):
                """Emit main DR matmuls for (pair g, out-tile jt)."""
                bs = (2 * g, 2 * g + 1)
                xps = [psX.tile([128, Tv], F32, tag="psX", name=f"xp{g}_{jt}_{i}")
                       for i in range(2)]
                for q in range(2):
                    for i, b in enumerate(bs):
                        nc.tensor.matmul(
                            xps[i][:],
                            wavS_sb[:, 2 * q:2 * q + 2, jt * 128:(jt + 1) * 128],
                            fT_sb[b][:, 2 * q:2 * q + 2, :],
                            start=(q == 0), stop=(q == 1), perf_mode=DR,
                        )
                return xps

            def frames_tanh(g, jt, xps):
                if (g, 0) not in yv_tiles:
                    for qq in range(2):
                        yv_tiles[(g, qq)] = work.tile(
                            [128, 2, 2, Tv], FP8, tag="yv4", name=f"yv4_{g}_{qq}",
                            bufs=4,
                        )
                q, jj = jt // 2, jt % 2
                for i, b in enumerate((2 * g, 2 * g + 1)):
                    nc.scalar.activation(
                        yv_tiles[(g, q)][:, jj, i, :], xps[i][:], ACT.Tanh,
                        bias=uhvbT_sb[:, jt * BL + b: jt * BL + b + 1],
                        scale=R64,
                    )

            def scv_mm(g, q):
                """Score DR matmuls for jt-pair q of pair g."""
                if g not in scv_tiles:
                    scv_tiles[g] = psS.tile([2, Tv], F32, tag="psS", name=f"scv{g}")
                scv_g = scv_tiles[g]
                yv4 = yv_tiles[(g, q)]
                for i in range(2):
                    nc.tensor.matmul(
                        scv_g[:], vavZ_sb[:, 2 * q:2 * q + 2, 2 * i:2 * i + 2],
                        yv4[:, :, i, :],
                        start=(q == 0 and i == 0), stop=(q == 1 and i == 1),
                        perf_mode=DR, skip_group_check=True,
                    )

            def pair_chain(g):
                """softmax -> DRAM-broadcast -> einsum for pair g."""
                bs = (2 * g, 2 * g + 1)
                scv_g = scv_tiles[g]
                expv = small.tile([2, Tv], F32, tag="expv", name=f"expv{g}", bufs=2)
                sumv = small.tile([2, 1], F32, tag="sumv", name=f"sumv{g}", bufs=2)
                nc.scalar.activation(
                    expv[:], scv_g[:], ACT.Exp, scale=R64, accum_out=sumv[:],
                )
                rv = small.tile([2, 1], F32, tag="rv", name=f"rv{g}", bufs=2)
                nc.vector.reciprocal(rv[:], sumv[:])
                avp = small.tile([2, Tv], BF16, tag="av", name=f"av{g}", bufs=2)
                nc.vector.tensor_scalar_mul(avp[:], expv[:], rv[:])
                avR = small.tile([1, 2 * Tv], BF16, tag="avR", name=f"avR{g}", bufs=2)
                nc.sync.dma_start(avR[0:1, :], avp[:, :])
                scr = work.tile([128, Tv], BF16, tag="scr", name=f"scr{g}", bufs=2)
                for i, b in enumerate(bs):
                    avB_ps = psB.tile([128, Tv], F32, tag="psB", name=f"avB{b}")
                    bsrc = avp[0:1, :] if i == 0 else avR[0:1, Tv:2 * Tv]
                    nc.tensor.matmul(
                        avB_ps[:], ones_sb[0:1, 0:128], bsrc,
                        start=True, stop=True, skip_group_check=True,
                    )
                    for kt in range(KT):
                        nc.vector.scalar_tensor_tensor(
                            out=scr[:],
                            in0=avB_ps[:],
                            scalar=1.0,
                            in1=fT_sb[b][:, kt, :],
                            op0=ALU.mult, op1=ALU.mult,
                            accum_out=hvT_sb[:, kt * BL + b: kt * BL + b + 1],
                        )

            # ---------- frames pairs 0-2 (earliest DMA arrivals) ----------
            for g in range(3):
                for jt in range(KT):
                    xps = frames_stage(g, jt)
                    if g >= 1 and jt == 0:
                        scv_mm(g - 1, 0)
                    elif g >= 1 and jt == 2:
                        scv_mm(g - 1, 1)
                        pair_chain(g - 1)
                    frames_tanh(g, jt, xps)

            # ---------- text stream (fp8 DR): Xt' = 64*(Wat.T@tT + Uht + bat)
            yt4 = small.tile([128, KT, BL * Tt], FP8, tag="yt4")
            for jt in range(KT):
                xt_ps = psX.tile([128, BL * Tt], F32, tag="psX", name=f"xt{jt}")
                for q in range(2):
                    nc.tensor.matmul(
                        xt_ps[:],
                        watS_sb[:, 2 * q:2 * q + 2, jt * 128:(jt + 1) * 128],
                        tT_sb[:, 2 * q:2 * q + 2, :],
                        start=(q == 0), stop=False, perf_mode=DR,
                        skip_group_check=True,
                    )
                nc.tensor.matmul(      # + 64*(Uht+bat) via block identity
                    xt_ps[:], uhtb_s[0:BL, jt * 128:(jt + 1) * 128], blkI_sb[:],
                    start=False, stop=True, skip_group_check=True,
                )
                nc.scalar.activation(yt4[:, jt, :], xt_ps[:], ACT.Tanh, scale=R64)

            scv_mm(2, 0)

            # hWhh + bh (bf16)
            hwhh_ps = gate16(hT16_sb, whh_sb, bh_r, name="hwhh")
            hwhh_sb = small.tile([BL, H], F32, tag="hwhh_sb")
            nc.scalar.copy(hwhh_sb[:], hwhh_ps[:])

            # sct: scores_t = 64 * yt @ Vat   (fp8 DR)
            sct_ps = psS.tile([1, BL * Tt], F32, tag="psS", name="sct")
            for q in range(2):
                nc.tensor.matmul(
                    sct_ps[:], vatZ_sb[:, 2 * q:2 * q + 2, 0:1],
                    yt4[:, 2 * q:2 * q + 2, :],
                    start=(q == 0), stop=(q == 1), perf_mode=DR,
                    skip_group_check=True,
                )

            # ---------- text softmax ----------
            sct_sb = small.tile([1, BL * Tt], F32, tag="sct_sb")
            nc.scalar.copy(sct_sb[:], sct_ps[:])
            st8 = small.tile([BL, Tt], F32, tag="st8")
            nc.sync.dma_start(st8[:, :], sct_sb[0:1, :])
            expt_sb = small.tile([BL, Tt], F32, tag="expt")
            sumt = small.tile([BL, 1], F32, tag="sumt")
            nc.scalar.activation(
                expt_sb[:], st8[:], ACT.Exp, scale=R64, accum_out=sumt[:],
            )
            rt = small.tile([BL, 1], F32, tag="rt")
            nc.vector.reciprocal(rt[:], sumt[:])
            at_sb = small.tile([BL, Tt], BF16, tag="at")
            nc.vector.tensor_scalar_mul(at_sb[:], expt_sb[:], rt[:])

            # atB: broadcast at across partitions via PE ones-matmul
            atRows = small.tile([1, BL * Tt], BF16, tag="atRows")
            nc.sync.dma_start(atRows[0:1, :], at_sb[:, :])
            atB_ps = psB.tile([128, BL * Tt], F32, tag="psB", name="atB")
            for b in range(BL):
                bsrc = at_sb[0:1, :] if b == 0 else atRows[0:1, b * Tt:(b + 1) * Tt]
                nc.tensor.matmul(
                    atB_ps[:, b * Tt:(b + 1) * Tt], ones_sb[0:1, 0:128], bsrc,
                    start=True, stop=True, skip_group_check=True,
                )
            atB_sb = small.tile([128, BL * Tt], BF16, tag="atB_sb")
            nc.vector.tensor_copy(atB_sb[:], atB_ps[:])

            # text einsum on DVE: htT[:, kt*8+b] = sum_t tT*at
            htT_sb = small.tile([128, KT * BL], F32, tag="htT")
            scrt = small.tile([128, Tt], BF16, tag="scrt")
            for kt in range(KT):
                for b in range(BL):
                    nc.vector.scalar_tensor_tensor(
                        out=scrt[:],
                        in0=tT_sb[:, kt, b * Tt:(b + 1) * Tt],
                        scalar=1.0,
                        in1=atB_sb[:, b * Tt:(b + 1) * Tt],
                        op0=ALU.mult, op1=ALU.mult,
                        accum_out=htT_sb[:, kt * BL + b: kt * BL + b + 1],
                    )

            # ---------- text gates + lgt into cc_in (all x1024-scaled) -------
            # wbs = 64*(h@Wb)  (shared by mt1 and mv1)
            wbs_ps = psG.tile([BL, H], F32, tag="psG", name="wbs")
            for q in range(2):
                nc.tensor.matmul(
                    wbs_ps[:], hT8_sb[:, 2 * q:2 * q + 2, 0:BL],
                    wb8_sb[:, 2 * q:2 * q + 2, :],
                    start=(q == 0), stop=(q == 1), perf_mode=DR,
                    skip_group_check=True,
                )
            wbs_sb = small.tile([BL, H], BF16, tag="wbs_sb")
            nc.scalar.copy(wbs_sb[:], wbs_ps[:])

            def gate1024(lhsT8, w8, bias_row, name):
                """x1024 group: 16*eyeD@wbs(64x) + bias(1024x) + lhsT8@w8."""
                ps = psG.tile([BL, H], F32, tag="psG", name=name)
                nc.tensor.matmul(
                    ps[:], eyeD_sb[:], wbs_sb[:],
                    start=True, stop=False, skip_group_check=True,
                )
                nc.tensor.matmul(
                    ps[:], ones_sb[0:1, 0:BL], bias_row,
                    start=False, stop=False, skip_group_check=True,
                )
                for q in range(2):
                    nc.tensor.matmul(
                        ps[:], lhsT8[:, 2 * q:2 * q + 2, 0:BL],
                        w8[:, 2 * q:2 * q + 2, :],
                        start=False, stop=(q == 1), perf_mode=DR,
                        skip_group_check=True,
                    )
                return ps

            def gate1024_nb(lhsT8, w8, bias_row, name):
                """x1024 group without wbs: bias(1024x) + lhsT8@w8."""
                ps = psG.tile([BL, H], F32, tag="psG", name=name)
                nc.tensor.matmul(
                    ps[:], ones_sb[0:1, 0:BL], bias_row,
                    start=True, stop=False, skip_group_check=True,
                )
                for q in range(2):
                    nc.tensor.matmul(
                        ps[:], lhsT8[:, 2 * q:2 * q + 2, 0:BL],
                        w8[:, 2 * q:2 * q + 2, :],
                        start=False, stop=(q == 1), perf_mode=DR,
                        skip_group_check=True,
                    )
                return ps

            # htT8: 16*ht_sum in fp8, padded [128, KT, 16]
            htT8 = small.tile([128, KT, 16], FP8, tag="htT8")
            for kt in range(KT):
                nc.vector.tensor_scalar_mul(
                    htT8[:, kt, 0:BL], htT_sb[:, kt * BL:(kt + 1) * BL], 16.0,
                )
            mt1_ps = gate1024(htT8, vbt8_sb, bbt_r, "mt1")
            ht2_ps = gate1024_nb(htT8, wqeT8_sb, bqe_r, "ht2")

            scv_mm(2, 1)
            pair_chain(2)

            lgt = small.tile([BL, 1], F32, tag="lgt")
            g_sb = small.tile([1, 2 * B], F32, tag="g")
            cc_in = dram.tile([2 * BL, 1], F32, tag="ccin")
            cc_out = dram.tile([2 * B, 1], F32, tag="ccout", addr_space="Shared")

            mtv_t = small.tile([BL, H], F32, tag="mtv_t")
            nc.scalar.activation(mtv_t[:], mt1_ps[:], ACT.Tanh, scale=1.0 / 1024.0)
            scr8b = small.tile([BL, H], F32, tag="scr8b")
            nc.vector.scalar_tensor_tensor(
                out=scr8b[:], in0=mtv_t[:], scalar=1.0, in1=wbB_sb[:],
                op0=ALU.mult, op1=ALU.mult, accum_out=lgt[:],
            )
            nc.sync.dma_start(cc_in[BL:2 * BL, :], lgt[:])
            # evacuate ht2 early (frees its PSUM bank before mv1/hv2)
            ht2_sb = small.tile([BL, H], F32, tag="ht2_sb")
            nc.scalar.mul(ht2_sb[:], ht2_ps[:], 1.0 / 1024.0)

            # ---------- frames pair 3 ----------
            for g in range(3, NP_):
                for jt in range(KT):
                    xps = frames_stage(g, jt)
                    frames_tanh(g, jt, xps)

            scv_mm(NP_ - 1, 0)
            scv_mm(NP_ - 1, 1)
            pair_chain(NP_ - 1)

            # ---------- visual gates (fp8, x1024) ----------
            hvT8 = small.tile([128, KT, 16], FP8, tag="hvT8")
            for kt in range(KT):
                nc.vector.tensor_scalar_mul(
                    hvT8[:, kt, 0:BL], hvT_sb[:, kt * BL:(kt + 1) * BL], 16.0,
                )
            mv1_ps = gate1024(hvT8, vbv8_sb, bbv_r, "mv1")
            hv2_ps = gate1024_nb(hvT8, wveT8_sb, bve_r, "hv2")

            mtv_v = small.tile([BL, H], F32, tag="mtv_v")
            nc.scalar.activation(mtv_v[:], mv1_ps[:], ACT.Tanh, scale=1.0 / 1024.0)
            lgv = small.tile([BL, 1], F32, tag="lgv")
            scr8 = small.tile([BL, H], F32, tag="scr8")
            nc.vector.scalar_tensor_tensor(
                out=scr8[:], in0=mtv_v[:], scalar=1.0, in1=wbB_sb[:],
                op0=ALU.mult, op1=ALU.mult, accum_out=lgv[:],
            )

            nc.sync.dma_start(cc_in[0:BL, :], lgv[:])
            nc.gpsimd.collective_compute(
                "AllGather", ALU.bypass,
                replica_groups=[list(range(NC))],
                ins=[cc_in[:].opt()],
                outs=[cc_out[:].opt()],
            )

            # ---------- global beta softmax + output ----------
            # g_sb layout: [core0: lgv(8), lgt(8); core1: ...] — order is
            # irrelevant for the softmax sum; beta0/1 = exp(g[0:2])/sum.
            nc.sync.dma_start(g_sb[0:1, :], cc_out[:, :])
            ge_sb = small.tile([1, 2 * B], F32, tag="ge")
            sumg = small.tile([1, 1], F32, tag="sumg")
            nc.scalar.activation(ge_sb[:], g_sb[:], ACT.Exp, accum_out=sumg[:])
            rg = small.tile([1, 1], F32, tag="rg")
            nc.vector.reciprocal(rg[:], sumg[:])
            betas = small.tile([1, 2], BF16, tag="betas")
            nc.vector.tensor_scalar_mul(betas[:], ge_sb[0:1, 0:2], rg[:])
            beta8_ps = psB.tile([BL, 2], F32, tag="psB", name="beta8")
            nc.tensor.matmul(
                beta8_ps[:], ones_sb[0:1, 0:BL], betas[0:1, 0:2],
                start=True, stop=True, skip_group_check=True,
            )
            hv2_sb = small.tile([BL, H], F32, tag="hv2_sb")
            nc.scalar.mul(hv2_sb[:], hv2_ps[:], 1.0 / 1024.0)
            t1 = small.tile([BL, H], F32, tag="t1")
            nc.vector.scalar_tensor_tensor(
                out=t1[:], in0=hv2_sb[:], scalar=beta8_ps[:, 0:1], in1=hwhh_sb[:],
                op0=ALU.mult, op1=ALU.add,
            )
            s1 = small.tile([BL, H], F32, tag="s1")
            nc.vector.scalar_tensor_tensor(
                out=s1[:], in0=ht2_sb[:], scalar=beta8_ps[:, 1:2], in1=t1[:],
                op0=ALU.mult, op1=ALU.add,
            )
            out_sb = small.tile([BL, H], F32, tag="out_sb")
            nc.scalar.activation(out_sb[:], s1[:], ACT.Tanh)
            nc.sync.dma_start(out_ext, out_sb[:])

    nc.compile()
    return nc


_cached_nc = None


def _get_nc():
    global _cached_nc
    if _cached_nc is None:
        _cached_nc = build_nc()
    return _cached_nc


def _pack_w8(w, scale=WS):
    """[512,512] -> [128, KT, H] fp8, [p, kt, j] = scale*w[kt*128+p, j]."""
    a = np.asarray(w, np.float32).reshape(KT, 128, H).transpose(1, 0, 2) * scale
    return np.ascontiguousarray(a).astype(NP_FP8)


def _pack_w16(w):
    a = np.asarray(w, np.float32).reshape(KT, 128, H).transpose(1, 0, 2)
    return np.ascontiguousarray(a.reshape(128, KT * H)).astype(NP_BF16)


def make_in_maps(inputs):
    h = np.asarray(inputs["h"], np.float32)
    frames = np.asarray(inputs["hidden_frames"], np.float32)
    text = np.asarray(inputs["hidden_text"], np.float32)
    Vav = np.asarray(inputs["Vav"], np.float32)
    Vat = np.asarray(inputs["Vat"], np.float32)
    wb = np.asarray(inputs["wb"], np.float32)

    vavZ = np.zeros((128, KT, 16), np.float32)
    for jt in range(KT):
        for i in range(2):
            vavZ[:, jt, 2 * i + i] = WS * Vav[jt * 128:(jt + 1) * 128]
    vavZ = vavZ.astype(NP_FP8)
    vatZ = np.zeros((128, KT, 16), np.float32)
    vatZ[:, :, 0] = (WS * Vat).reshape(KT, 128).T
    vatZ = vatZ.astype(NP_FP8)

    biasr = np.zeros((1, 8 * H), np.float32)
    for i, (k, sc) in enumerate([
        ("bav", WS), ("bat", WS), ("bve", 1024.0), ("bqe", 1024.0),
        ("bbv", 1024.0), ("bbt", 1024.0), ("bh", 1.0),
    ]):
        biasr[0, i * H:(i + 1) * H] = sc * np.asarray(inputs[k], np.float32)
    biasr = biasr.astype(NP_BF16)
    wbB = np.ascontiguousarray(np.broadcast_to(wb, (BL, H))).astype(np.float32)
    eye = np.eye(128, dtype=np.float32).astype(NP_BF16)
    blkI = np.zeros((BL, BL, Tt), np.float32)
    for b in range(BL):
        blkI[b, b, :] = 1.0
    blkI = blkI.reshape(BL, BL * Tt).astype(NP_BF16)

    eyeD = (16.0 * np.eye(BL, dtype=np.float32)).astype(NP_BF16)
    shared = dict(
        wavS=_pack_w8(inputs["Wav"]), watS=_pack_w8(inputs["Wat"]),
        uavS=_pack_w8(inputs["Uav"]), uatS=_pack_w8(inputs["Uat"]),
        wb8=_pack_w8(inputs["Wb"]), whh=_pack_w16(inputs["Whh"]),
        wveT8=_pack_w8(np.asarray(inputs["Wve"], np.float32).T),
        wqeT8=_pack_w8(np.asarray(inputs["Wqe"], np.float32).T),
        vbv8=_pack_w8(inputs["Vbv"]), vbt8=_pack_w8(inputs["Vbt"]),
        vavZ=vavZ, vatZ=vatZ, biasr=biasr, wbB=wbB, eye=eye, blkI=blkI,
        eyeD=eyeD,
    )

    in_maps = []
    for i in range(NC):
        sl = slice(i * BL, (i + 1) * BL)
        fTc = np.ascontiguousarray(
            frames[sl].transpose(0, 2, 1)           # [BL, H, Tv]
            .reshape(BL, KT, 128, Tv)
            .transpose(0, 2, 1, 3)                  # [BL, 128, KT, Tv]
        ).astype(NP_FP8)
        tTc = np.ascontiguousarray(
            text[sl].transpose(2, 0, 1)             # [H, BL, Tt]
            .reshape(KT, 128, BL, Tt)
            .transpose(1, 0, 2, 3)                  # [128, KT, BL, Tt]
            .reshape(128, KT, BL * Tt)
        ).astype(NP_FP8)
        hTc = np.ascontiguousarray(
            h[sl].T.reshape(KT, 128, BL).transpose(1, 0, 2)
        )
        hT8c = np.zeros((128, KT, 16), np.float32)
        hT8c[:, :, 0:BL] = hTc
        in_maps.append(dict(
            shared, fT=fTc, tT=tTc,
            hT8=hT8c.astype(NP_FP8),
            hT16=hTc.reshape(128, KT * BL).astype(NP_BF16),
        ))
    return in_maps


def run(inputs, trace=False, **kw):
    nc = _get_nc()
    in_maps = make_in_maps(inputs)
    res = run_bass_kernel_spmd(nc, in_maps, core_ids=list(range(NC)), trace=trace, **kw)
    out = np.concatenate([res.results[i]["out"] for i in range(NC)], axis=0)
    return out, res


def kernel(**inputs) -> np.ndarray:
    out, _ = run(inputs, trace=False)
    return out
